# revision 18
# baseline (speedup 1.0000x reference)
"""DAGNN (GRU + 2xGAT + mean-pool + fc + log_softmax) on 8 TRN2 cores via Bass/Tile.

Sharding: nodes split across cores by dst-range (edges sorted by dst, split at
dst boundaries), so each core's GRU computes exactly the h/attention values its
GAT dst windows need locally. Edge payload gathers use batched dma_gather from
a 4-way row-sharded bf16 table (int16 index limit); per-window dst attention
terms are expanded on-chip via selection-matrix matmuls. Feature tables are
AllGathered; graph pooling partial sums are AllReduced.
"""
import sys
import numpy as np

sys.path.insert(0, "/opt/trn_rl_repo")

import ml_dtypes
import concourse.bass as bass
import concourse.bacc as bacc
import concourse.mybir as mybir
import concourse.tile as tile
from concourse.masks import make_identity

F32 = mybir.dt.float32
BF16 = mybir.dt.bfloat16
I16 = mybir.dt.int16
U8 = mybir.dt.uint8
AF = mybir.ActivationFunctionType
OP = mybir.AluOpType

NSH = 4          # table row shards (int16 gather index limit)
GT = 512         # GRU node tile


def _ceil(a, b):
    return -(-a // b)


class Cfg:
    def __init__(self, N, E, G, P):
        self.N, self.E, self.G, self.P = N, E, G, P
        self.T, self.D, self.H = 8, 128, 64
        self.HEADS, self.C1, self.C2 = 4, 256, 64


def host_prep(cfg, edge_index, batch):
    N, E, P = cfg.N, cfg.E, cfg.P
    src = np.concatenate([np.asarray(edge_index[0], np.int64), np.arange(N, dtype=np.int64)])
    dst = np.concatenate([np.asarray(edge_index[1], np.int64), np.arange(N, dtype=np.int64)])
    order = np.argsort(dst, kind="stable")
    ss, dd = src[order], dst[order]
    Etot = ss.shape[0]

    bounds = [0]
    for k in range(1, P):
        pos = (k * Etot) // P
        while pos < Etot and dd[pos] == dd[pos - 1]:
            pos += 1
        bounds.append(pos)
    bounds.append(Etot)
    n0 = np.zeros(P + 1, np.int64)
    n0[P] = N
    for c in range(1, P):
        n0[c] = dd[bounds[c]]
    ranges = np.diff(n0)
    NPAD2 = _ceil(int(ranges.max()), GT) * GT
    NW = NPAD2 // 128
    SH = (P * NPAD2) // NSH
    assert SH - 1 <= 32767, f"shard too large for int16: {SH}"
    cfg.n0, cfg.NPAD2, cfg.NW, cfg.SH = n0, NPAD2, NW, SH
    cfg.NT = NPAD2 // GT

    owner = np.searchsorted(n0[1:P], np.arange(N), side="right")
    g2r = owner * NPAD2 + (np.arange(N) - n0[owner])
    shard_of = (g2r // SH).astype(np.int64)
    rel_of = (g2r % SH).astype(np.int16)

    # pass 1: per-(window, shard) edge counts per core -> uniform tile counts
    NB = NW * NSH
    kws = np.zeros((P, NB), np.int64)
    per_edges = []
    for c in range(P):
        sl = slice(bounds[c], bounds[c + 1])
        ssc, ddc = ss[sl], dd[sl]
        w_arr = (ddc - n0[c]) // 128
        s_arr = shard_of[ssc]
        key = (w_arr * NSH + s_arr).astype(np.int64)
        kws[c] = np.bincount(key, minlength=NB)
        per_edges.append((ssc, ddc, w_arr, key))
    tiles = np.maximum(1, _ceil(kws.max(axis=0), 128)).astype(np.int64)
    tile_off = np.concatenate([[0], np.cumsum(tiles)])
    TOT_TILES = int(tile_off[-1])
    cfg.tiles, cfg.tile_off, cfg.TOT_TILES = tiles, tile_off, TOT_TILES
    cfg.TBMAX = int(tiles.max())

    per_core = []
    for c in range(P):
        ssc, ddc, w_arr, key = per_edges[c]
        order2 = np.argsort(key, kind="stable")
        sk = key[order2]
        grp_start = np.searchsorted(sk, np.arange(NB))
        rank = np.arange(sk.shape[0]) - grp_start[sk]
        slotpos = tile_off[sk] * 128 + rank
        TOT_SLOT = TOT_TILES * 128
        srel = np.zeros(TOT_SLOT, np.int16)
        drel = np.full(TOT_SLOT, 255, np.uint8)
        srel[slotpos] = rel_of[ssc[order2]]
        drel[slotpos] = (ddc[order2] - n0[c] - 128 * w_arr[order2]).astype(np.uint8)
        # wrapped gather indices: idx i of a (tile-aligned) run at [i%16, i//16]
        wr = np.ascontiguousarray(srel.reshape(TOT_SLOT // 16, 16).T)
        idx_wr = np.tile(wr, (8, 1))                                   # [128, TOT_SLOT//16]
        drel_pt = np.ascontiguousarray(drel.reshape(TOT_TILES, 128).T)  # [128, TOT_TILES]
        drelT = drel.reshape(1, TOT_TILES, 128).copy()                  # [1, TOT_TILES, 128]
        bd = np.full(NPAD2, 999.0, np.float32)
        rg = int(ranges[c])
        bd[:rg] = np.asarray(batch, np.int64)[n0[c]:n0[c + 1]].astype(np.float32)
        batch_wd = np.ascontiguousarray(bd.reshape(NW, 128).T)          # [128, NW]
        per_core.append(dict(idx_wr=idx_wr, drel_pt=drel_pt, drelT=drelT,
                             batch_wd=batch_wd, rg=rg))
    return per_core


def build_inputs(cfg, x, weights, per_core):
    (gru_w_ih, gru_w_hh, gru_b_ih, gru_b_hh, W1, att_src1, att_dst1, b1,
     W2, att_src2, att_dst2, b2, fc_w, fc_b) = weights
    P, NPAD2 = cfg.P, cfg.NPAD2
    bf = ml_dtypes.bfloat16

    # BC1 [64, 8]: cols 0:4 src-att coeffs per head, 4:8 dst-att
    BC1 = np.zeros((64, 8), np.float32)
    for h in range(4):
        Wh = W1[:, 64 * h:64 * (h + 1)]
        BC1[:, h] = Wh @ att_src1[h]
        BC1[:, 4 + h] = Wh @ att_dst1[h]
    # W1 block-diagonal pairs for transposed apply
    W12A = np.zeros((128, 128), np.float32)
    W12B = np.zeros((128, 128), np.float32)
    W12A[0:64, 0:64] = W1[:, 0:64]
    W12A[64:128, 64:128] = W1[:, 64:128]
    W12B[0:64, 0:64] = W1[:, 128:192]
    W12B[64:128, 64:128] = W1[:, 192:256]
    # W2 halves side by side; BC2 [128, 4]: cols 2h = [src|dst] coeffs, half h
    W2s = np.zeros((128, 128), np.float32)
    W2s[:, 0:64] = W2[0:128, :]
    W2s[:, 64:128] = W2[128:256, :]
    a2 = W2 @ att_src2[0]   # [256]
    d2 = W2 @ att_dst2[0]
    BC2 = np.zeros((128, 4), np.float32)
    BC2[:, 0] = a2[0:128]
    BC2[:, 1] = d2[0:128]
    BC2[:, 2] = a2[128:256]
    BC2[:, 3] = d2[128:256]

    com = dict(
        wihT=np.ascontiguousarray(gru_w_ih.T).astype(bf),               # [128,192]
        whrz=np.concatenate([gru_w_hh.T[:, 0:128],
                             (gru_b_ih + gru_b_hh)[None, 0:128]], 0).astype(bf),  # [65,128]
        whn=np.concatenate([gru_w_hh.T[:, 128:192],
                            gru_b_hh[None, 128:192]], 0).astype(bf),    # [65,64]
        bihn=np.ascontiguousarray(gru_b_ih[128:192].reshape(64, 1)).astype(np.float32),
        BC1=BC1.astype(bf),
        W12A=W12A.astype(bf), W12B=W12B.astype(bf),
        b1b=np.broadcast_to(b1, (128, 256)).astype(bf).copy(),
        W2s=W2s.astype(bf), BC2=BC2.astype(bf),
        b2b=np.broadcast_to(b2, (128, 64)).astype(bf).copy(),
        fcw=fc_w.astype(np.float32),
        fcbb=np.broadcast_to(fc_b, (128, 10)).astype(np.float32).copy(),
    )
    in_maps = []
    for c in range(P):
        pc = per_core[c]
        rg = pc["rg"]
        xp = np.zeros((NPAD2, cfg.T, cfg.D), np.float32)
        xp[:rg] = x[cfg.n0[c]:cfg.n0[c + 1]]
        xpT = np.ascontiguousarray(xp.transpose(1, 2, 0)).astype(bf)     # [8,128,NPAD2]
        m = dict(com)
        m.update(xpT=xpT, idx_wr=pc["idx_wr"], drel_pt=pc["drel_pt"],
                 drelT=pc["drelT"], batch_wd=pc["batch_wd"])
        in_maps.append(m)
    return in_maps


def build_kernel(cfg, dbg=False):
    P, T, NPAD2, NW, SH = cfg.P, cfg.T, cfg.NPAD2, cfg.NW, cfg.SH
    NT, NSHARD = cfg.NT, NSH
    tiles, tile_off, TOT_TILES = cfg.tiles, cfg.tile_off, cfg.TOT_TILES
    TOT_SLOT = TOT_TILES * 128
    rg_all = [list(range(P))]

    nc = bacc.Bacc("TRN2", target_bir_lowering=False, debug=False,
                   dynamic_dma_scratch_size=32768)
    # inputs
    xpT = nc.dram_tensor("xpT", [T, 128, NPAD2], BF16, kind="ExternalInput")
    wihT = nc.dram_tensor("wihT", [128, 192], BF16, kind="ExternalInput")
    whrz = nc.dram_tensor("whrz", [65, 128], BF16, kind="ExternalInput")
    whn = nc.dram_tensor("whn", [65, 64], BF16, kind="ExternalInput")
    bihn = nc.dram_tensor("bihn", [64, 1], F32, kind="ExternalInput")
    BC1 = nc.dram_tensor("BC1", [64, 8], BF16, kind="ExternalInput")
    W12A = nc.dram_tensor("W12A", [128, 128], BF16, kind="ExternalInput")
    W12B = nc.dram_tensor("W12B", [128, 128], BF16, kind="ExternalInput")
    b1b = nc.dram_tensor("b1b", [128, 256], BF16, kind="ExternalInput")
    W2s = nc.dram_tensor("W2s", [128, 128], BF16, kind="ExternalInput")
    BC2 = nc.dram_tensor("BC2", [128, 4], BF16, kind="ExternalInput")
    b2b = nc.dram_tensor("b2b", [128, 64], BF16, kind="ExternalInput")
    fcw = nc.dram_tensor("fcw", [64, 10], F32, kind="ExternalInput")
    fcbb = nc.dram_tensor("fcbb", [128, 10], F32, kind="ExternalInput")
    idx_wr = nc.dram_tensor("idx_wr", [128, TOT_SLOT // 16], I16, kind="ExternalInput")
    drel_pt = nc.dram_tensor("drel_pt", [128, TOT_TILES], U8, kind="ExternalInput")
    drelT = nc.dram_tensor("drelT", [1, TOT_TILES, 128], U8, kind="ExternalInput")
    batch_wd = nc.dram_tensor("batch_wd", [128, NW], F32, kind="ExternalInput")
    out = nc.dram_tensor("out", [128, 16], F32, kind="ExternalOutput")
    # internal dram
    xcat1l = nc.dram_tensor("xcat1l", [NPAD2, 128], BF16)
    table1 = nc.dram_tensor("table1", [P * NPAD2, 128], BF16, addr_space="Shared")
    tb1s = [nc.dram_tensor(f"tb1s{s}", [SH, 128], BF16) for s in range(NSHARD)]
    xcat2l = nc.dram_tensor("xcat2l", [NPAD2, 128], BF16)
    table2 = nc.dram_tensor("table2", [P * NPAD2, 128], BF16, addr_space="Shared")
    tb2s = [nc.dram_tensor(f"tb2s{s}", [SH, 128], BF16) for s in range(NSHARD)]
    arin = nc.dram_tensor("arin", [128, 65], F32)
    arout = nc.dram_tensor("arout", [128, 65], F32, addr_space="Shared")

    with tile.TileContext(nc) as tc:
        with tc.tile_pool(name="pers", bufs=1) as pers:
            # ---- persistent: weights, indices, iotas ----
            identb = pers.tile([128, 128], BF16)
            make_identity(nc, identb[:])
            iota_row = pers.tile([128, 128], U8)
            nc.gpsimd.iota(iota_row[:], pattern=[[1, 128]], base=0, channel_multiplier=0,
                           allow_small_or_imprecise_dtypes=True)
            iota_p = pers.tile([128, 1], F32)
            nc.gpsimd.iota(iota_p[:], pattern=[[0, 1]], base=0, channel_multiplier=1,
                           allow_small_or_imprecise_dtypes=True)
            wihT_s = pers.tile([128, 192], BF16)
            nc.sync.dma_start(out=wihT_s[:], in_=wihT[:])
            whrz_s = pers.tile([65, 128], BF16)
            nc.sync.dma_start(out=whrz_s[:], in_=whrz[:])
            whn_s = pers.tile([65, 64], BF16)
            nc.sync.dma_start(out=whn_s[:], in_=whn[:])
            bihn_s = pers.tile([64, 1], F32)
            nc.sync.dma_start(out=bihn_s[:], in_=bihn[:])
            BC1_s = pers.tile([64, 8], BF16)
            nc.sync.dma_start(out=BC1_s[:], in_=BC1[:])
            W12A_s = pers.tile([128, 128], BF16)
            nc.sync.dma_start(out=W12A_s[:], in_=W12A[:])
            W12B_s = pers.tile([128, 128], BF16)
            nc.sync.dma_start(out=W12B_s[:], in_=W12B[:])
            b1b_s = pers.tile([128, 256], BF16)
            nc.sync.dma_start(out=b1b_s[:], in_=b1b[:])
            W2s_s = pers.tile([128, 128], BF16)
            nc.sync.dma_start(out=W2s_s[:], in_=W2s[:])
            BC2_s = pers.tile([128, 4], BF16)
            nc.sync.dma_start(out=BC2_s[:], in_=BC2[:])
            b2b_s = pers.tile([128, 64], BF16)
            nc.sync.dma_start(out=b2b_s[:], in_=b2b[:])
            fcw_s = pers.tile([64, 10], F32)
            nc.sync.dma_start(out=fcw_s[:], in_=fcw[:])
            fcbb_s = pers.tile([128, 10], F32)
            nc.sync.dma_start(out=fcbb_s[:], in_=fcbb[:])
            idx_sb = pers.tile([128, TOT_SLOT // 16], I16)
            nc.sync.dma_start(out=idx_sb[:], in_=idx_wr[:])
            drel_sb = pers.tile([128, TOT_TILES], U8)
            nc.sync.dma_start(out=drel_sb[:], in_=drel_pt[:])
            batch_sb = pers.tile([128, NW], F32)
            nc.sync.dma_start(out=batch_sb[:], in_=batch_wd[:])
            adt1_sb = pers.tile([128, NW, 4], BF16)
            adt2_sb = pers.tile([128, NW, 1], BF16)
            nidx_regs = {}
            for b in range(NW * NSHARD):
                Tb = int(tiles[b])
                for q0 in range(0, Tb, 8):
                    n = min(8, Tb - q0) * 128
                    if n not in nidx_regs:
                        nidx_regs[n] = nc.gpsimd.to_reg(n)

            # ---- phase 1: GRU -> xcat1l (h|asrc), adt1_sb ----
            with (
                tc.tile_pool(name="gx", bufs=2) as gx,
                tc.tile_pool(name="gh", bufs=2) as gh,
                tc.tile_pool(name="gv", bufs=3) as gv,
                tc.tile_pool(name="gp1", bufs=1, space="PSUM") as gp1,
                tc.tile_pool(name="gp2", bufs=1, space="PSUM") as gp2,
                tc.tile_pool(name="gp3", bufs=1, space="PSUM") as gp3,
                tc.tile_pool(name="gp4", bufs=2, space="PSUM") as gp4,
            ):
                for it in range(NT):
                    xt8 = gx.tile([128, T, GT], BF16, tag="xt8")
                    for t in range(T):
                        nc.sync.dma_start(out=xt8[:, t, :],
                                          in_=xpT[t, :, it * GT:(it + 1) * GT])
                    hT = gh.tile([65, GT], BF16, tag="hT")
                    nc.vector.memset(hT[0:64, :], 0.0)
                    nc.vector.memset(hT[64:65, :], 1.0)
                    for t in range(T):
                        prz = gp1.tile([64, 2 * GT], F32, tag="prz")
                        nc.tensor.matmul(out=prz[:, 0:GT], lhsT=wihT_s[:, 0:64],
                                         rhs=xt8[:, t, :], start=True, stop=False)
                        nc.tensor.matmul(out=prz[:, 0:GT], lhsT=whrz_s[:, 0:64],
                                         rhs=hT[:], start=False, stop=True)
                        nc.tensor.matmul(out=prz[:, GT:2 * GT], lhsT=wihT_s[:, 64:128],
                                         rhs=xt8[:, t, :], start=True, stop=False)
                        nc.tensor.matmul(out=prz[:, GT:2 * GT], lhsT=whrz_s[:, 64:128],
                                         rhs=hT[:], start=False, stop=True)
                        pin = gp2.tile([64, GT], F32, tag="pin")
                        nc.tensor.matmul(out=pin[:], lhsT=wihT_s[:, 128:192],
                                         rhs=xt8[:, t, :], start=True, stop=True)
                        phn = gp3.tile([64, GT], F32, tag="phn")
                        nc.tensor.matmul(out=phn[:], lhsT=whn_s[:], rhs=hT[:],
                                         start=True, stop=True)
                        rz = gv.tile([64, 2 * GT], BF16, tag="rz")
                        nc.scalar.activation(out=rz[:], in_=prz[:], func=AF.Sigmoid)
                        tmp = gv.tile([64, GT], BF16, tag="tmp")
                        nc.vector.tensor_mul(out=tmp[:], in0=rz[:, 0:GT], in1=phn[:])
                        t3 = gv.tile([64, GT], F32, tag="t3")
                        nc.vector.tensor_add(out=t3[:], in0=pin[:], in1=tmp[:])
                        nh = gv.tile([64, GT], BF16, tag="nh")
                        nc.scalar.activation(out=nh[:], in_=t3[:], func=AF.Tanh,
                                             bias=bihn_s[:, 0:1])
                        s1 = gv.tile([64, GT], BF16, tag="s1")
                        nc.gpsimd.tensor_sub(out=s1[:], in0=hT[0:64, :], in1=nh[:])
                        s2 = gv.tile([64, GT], BF16, tag="s2")
                        nc.vector.tensor_mul(out=s2[:], in0=rz[:, GT:2 * GT], in1=s1[:])
                        nc.vector.tensor_add(out=hT[0:64, :], in0=nh[:], in1=s2[:])
                    for cc in range(GT // 128):
                        w = it * (GT // 128) + cc
                        pt = gp4.tile([128, 64], BF16, tag="pt")
                        nc.tensor.transpose(out=pt[:],
                                            in_=hT[0:64, cc * 128:(cc + 1) * 128],
                                            identity=identb[0:64, 0:64])
                        pat = gp4.tile([128, 8], F32, tag="pat")
                        nc.tensor.matmul(out=pat[:], lhsT=hT[0:64, cc * 128:(cc + 1) * 128],
                                         rhs=BC1_s[:], start=True, stop=True)
                        xc = gv.tile([128, 128], BF16, tag="xc")
                        nc.scalar.copy(out=xc[:, 0:64], in_=pt[:])
                        nc.vector.tensor_copy(out=xc[:, 64:68], in_=pat[:, 0:4])
                        nc.vector.memset(xc[:, 68:128], 0.0)
                        nc.vector.tensor_copy(out=adt1_sb[:, w, :], in_=pat[:, 4:8])
                        nc.sync.dma_start(out=xcat1l[w * 128:(w + 1) * 128, :], in_=xc[:])

            # ---- AllGather table1, split into shards ----
            nc.gpsimd.collective_compute(
                "AllGather", OP.bypass, replica_groups=rg_all,
                ins=[xcat1l[:]], outs=[table1[:]])
            for s in range(NSHARD):
                for ci in range(2):
                    nc.sync.dma_start(
                        out=tb1s[s][ci * NPAD2:(ci + 1) * NPAD2, :],
                        in_=table1[(2 * s + ci) * NPAD2:(2 * s + ci + 1) * NPAD2, :])
            tc.strict_bb_all_engine_barrier()

            # ---- GAT layer over windows (shared for layer 1 / layer 2) ----
            def gat_windows(tbls, adt_sb, nheads, payw, post_fn):
                """payw: scatter matmul width (4+256 for L1, 1+64 for L2)."""
                with (
                    tc.tile_pool(name="pg", bufs=3) as pg,
                    tc.tile_pool(name="pS", bufs=2) as pS,
                    tc.tile_pool(name="pd", bufs=2) as pd,
                    tc.tile_pool(name="pu", bufs=2) as pu,
                    tc.tile_pool(name="pM", bufs=2) as pM,
                    tc.tile_pool(name="pw", bufs=2) as pw,
                    tc.tile_pool(name="ps2", bufs=2, space="PSUM") as ps2,
                    tc.tile_pool(name="psA", bufs=1, space="PSUM") as psA,
                    tc.tile_pool(name="psB", bufs=1, space="PSUM") as psB,
                ):
                    for w in range(NW):
                        pblk = ps2.tile([128, payw], F32, tag="pblk")
                        first = True
                        for s in range(NSHARD):
                            b = w * NSHARD + s
                            Tb = int(tiles[b])
                            t0 = int(tile_off[b])
                            g = pg.tile([128, Tb, 128], BF16, tag=f"g{Tb}")
                            for q0 in range(0, Tb, 8):
                                qn = min(8, Tb - q0)
                                nc.gpsimd.dma_gather(
                                    out_ap=g[:, q0:q0 + qn, :], in_ap=tbls[s][:],
                                    idxs_ap=idx_sb[:, (t0 + q0) * 8:(t0 + q0 + qn) * 8],
                                    num_idxs=qn * 128, num_idxs_reg=nidx_regs[qn * 128],
                                    elem_size=128)
                            S = pS.tile([128, Tb, 128], BF16, tag=f"S{Tb}")
                            nc.vector.tensor_tensor(
                                out=S[:],
                                in0=iota_row[:].unsqueeze(1).broadcast_to([128, Tb, 128]),
                                in1=drel_sb[:, t0:t0 + Tb].unsqueeze(2).broadcast_to([128, Tb, 128]),
                                op=OP.is_equal)
                            drT = pd.tile([128, Tb, 128], U8, tag=f"dT{Tb}")
                            nc.sync.dma_start(
                                out=drT[:],
                                in_=drelT[0:1, t0:t0 + Tb, :].partition_broadcast(128))
                            Sd = pS.tile([128, Tb, 128], BF16, tag=f"Sd{Tb}")
                            nc.vector.tensor_scalar(
                                out=Sd[:], in0=drT[:], scalar1=iota_p[:, 0:1],
                                scalar2=None, op0=OP.is_equal)
                            padp = psA.tile([128, Tb * nheads], F32, tag="padp")
                            for t in range(Tb):
                                nc.tensor.matmul(
                                    out=padp[:, t * nheads:(t + 1) * nheads],
                                    lhsT=Sd[:, t, :], rhs=adt_sb[:, w, :],
                                    start=True, stop=True)
                            u = pu.tile([128, Tb, nheads], F32, tag=f"u{Tb}")
                            nc.vector.tensor_add(
                                out=u[:], in0=g[:, :, 64:64 + nheads],
                                in1=padp[:].rearrange("p (t c) -> p t c", t=Tb))
                            e1 = pu.tile([128, Tb, nheads], BF16, tag=f"e1{Tb}")
                            nc.scalar.activation(out=e1[:], in_=u[:], func=AF.Exp)
                            e2 = pu.tile([128, Tb, nheads], BF16, tag=f"e2{Tb}")
                            nc.scalar.activation(out=e2[:], in_=u[:], func=AF.Exp, scale=0.2)
                            ee = pu.tile([128, Tb, nheads], BF16, tag=f"ee{Tb}")
                            nc.vector.tensor_tensor(out=ee[:], in0=e1[:], in1=e2[:], op=OP.max)
                            M = pM.tile([128, Tb, payw], BF16, tag=f"M{Tb}")
                            nc.vector.tensor_copy(out=M[:, :, 0:nheads], in_=ee[:])
                            if nheads == 4:
                                nc.vector.tensor_tensor(
                                    out=M[:, :, 4:260].rearrange("p t (h c) -> p t h c", h=4),
                                    in0=g[:, :, 0:64].unsqueeze(2).broadcast_to([128, Tb, 4, 64]),
                                    in1=ee[:].unsqueeze(3).broadcast_to([128, Tb, 4, 64]),
                                    op=OP.mult)
                            else:
                                nc.vector.tensor_tensor(
                                    out=M[:, :, 1:65],
                                    in0=g[:, :, 0:64],
                                    in1=ee[:].to_broadcast([128, Tb, 64]),
                                    op=OP.mult)
                            for t in range(Tb):
                                nc.tensor.matmul(
                                    out=pblk[:], lhsT=S[:, t, :], rhs=M[:, t, :],
                                    start=first, stop=(s == NSHARD - 1 and t == Tb - 1))
                                first = False
                        post_fn(w, pblk, pw, psB)

            # ---- layer 1 post: h1 = relu(z/denom @ W1 + b1) -> xcat2, adt2 ----
            def post1(w, pblk, pw, psB):
                dn = pw.tile([128, 4], F32, tag="dn")
                nc.vector.tensor_scalar(out=dn[:], in0=pblk[:, 0:4], scalar1=1e-12,
                                        scalar2=None, op0=OP.max)
                rec = pw.tile([128, 4], F32, tag="rec")
                nc.vector.reciprocal(out=rec[:], in_=dn[:])
                zn = pw.tile([128, 256], BF16, tag="zn")
                nc.vector.tensor_tensor(
                    out=zn[:].rearrange("p (h c) -> p h c", h=4),
                    in0=pblk[:, 4:260].rearrange("p (h c) -> p h c", h=4),
                    in1=rec[:].unsqueeze(2).broadcast_to([128, 4, 64]),
                    op=OP.mult)
                tpz = pw.tile([128, 256], BF16, tag="tpz")
                for half in range(2):
                    ptp = psB.tile([128, 128], BF16, tag="ptp")
                    nc.tensor.transpose(out=ptp[:], in_=zn[:, 128 * half:128 * (half + 1)],
                                        identity=identb[:])
                    nc.scalar.copy(out=tpz[:, 128 * half:128 * (half + 1)], in_=ptp[:])
                h1p = psB.tile([128, 256], F32, tag="h1p")
                nc.tensor.matmul(out=h1p[:, 0:128], lhsT=tpz[:, 0:128], rhs=W12A_s[:],
                                 start=True, stop=True)
                nc.tensor.matmul(out=h1p[:, 128:256], lhsT=tpz[:, 128:256], rhs=W12B_s[:],
                                 start=True, stop=True)
                h1 = pw.tile([128, 256], BF16, tag="h1")
                nc.vector.tensor_add(out=h1[:], in0=h1p[:], in1=b1b_s[:])
                nc.vector.tensor_scalar(out=h1[:], in0=h1[:], scalar1=0.0,
                                        scalar2=None, op0=OP.max)
                th1 = pw.tile([128, 256], BF16, tag="th1")
                for half in range(2):
                    ptp = psB.tile([128, 128], BF16, tag="ptp")
                    nc.tensor.transpose(out=ptp[:], in_=h1[:, 128 * half:128 * (half + 1)],
                                        identity=identb[:])
                    nc.scalar.copy(out=th1[:, 128 * half:128 * (half + 1)], in_=ptp[:])
                xsc = psB.tile([128, 66], F32, tag="xsc")
                xs2p = xsc[:, 0:64]
                pat2 = xsc[:, 64:66]
                for half in range(2):
                    nc.tensor.matmul(out=xs2p, lhsT=th1[:, 128 * half:128 * (half + 1)],
                                     rhs=W2s_s[:, 64 * half:64 * (half + 1)],
                                     start=(half == 0), stop=(half == 1))
                    nc.tensor.matmul(out=pat2, lhsT=th1[:, 128 * half:128 * (half + 1)],
                                     rhs=BC2_s[:, 2 * half:2 * half + 2],
                                     start=(half == 0), stop=(half == 1))
                xc2 = pw.tile([128, 128], BF16, tag="xc2")
                nc.scalar.copy(out=xc2[:, 0:64], in_=xs2p)
                nc.vector.tensor_copy(out=xc2[:, 64:65], in_=pat2[:, 0:1])
                nc.vector.memset(xc2[:, 65:128], 0.0)
                nc.vector.tensor_copy(out=adt2_sb[:, w, :], in_=pat2[:, 1:2])
                nc.sync.dma_start(out=xcat2l[w * 128:(w + 1) * 128, :], in_=xc2[:])

            gat_windows(tb1s, adt1_sb, 4, 260, post1)

            nc.gpsimd.collective_compute(
                "AllGather", OP.bypass, replica_groups=rg_all,
                ins=[xcat2l[:]], outs=[table2[:]])
            for s in range(NSHARD):
                for ci in range(2):
                    nc.sync.dma_start(
                        out=tb2s[s][ci * NPAD2:(ci + 1) * NPAD2, :],
                        in_=table2[(2 * s + ci) * NPAD2:(2 * s + ci + 1) * NPAD2, :])
            tc.strict_bb_all_engine_barrier()

            # ---- layer 2 post: pooling into ppool psum ----
            pp_ctx = tc.tile_pool(name="psPool", bufs=1, space="PSUM")
            psPool = pp_ctx.__enter__()
            ppool = psPool.tile([128, 65], F32)

            def post2(w, pblk, pw, psB):
                dn2 = pw.tile([128, 1], F32, tag="dn2")
                nc.vector.tensor_scalar(out=dn2[:], in0=pblk[:, 0:1], scalar1=1e-12,
                                        scalar2=None, op0=OP.max)
                rec2 = pw.tile([128, 1], F32, tag="rec2")
                nc.vector.reciprocal(out=rec2[:], in_=dn2[:])
                ph = pw.tile([128, 65], BF16, tag="ph")
                nc.vector.tensor_scalar(out=ph[:, 0:64], in0=pblk[:, 1:65],
                                        scalar1=rec2[:, 0:1], scalar2=None, op0=OP.mult)
                nc.vector.tensor_add(out=ph[:, 0:64], in0=ph[:, 0:64], in1=b2b_s[:])
                nc.vector.tensor_scalar(out=ph[:, 0:64], in0=ph[:, 0:64], scalar1=0.0,
                                        scalar2=None, op0=OP.max)
                nc.vector.memset(ph[:, 64:65], 1.0)
                Sb = pw.tile([128, 128], BF16, tag="Sb")
                nc.vector.tensor_scalar(out=Sb[:], in0=iota_row[:],
                                        scalar1=batch_sb[:, w:w + 1], scalar2=None,
                                        op0=OP.is_equal)
                nc.tensor.matmul(out=ppool[:], lhsT=Sb[:], rhs=ph[:],
                                 start=(w == 0), stop=(w == NW - 1))

            gat_windows(tb2s, adt2_sb, 1, 65, post2)

            # ---- tail: AllReduce pools, fc, log_softmax ----
            with tc.tile_pool(name="rpre", bufs=1) as rpre:
                pr = rpre.tile([128, 65], F32)
                nc.scalar.copy(out=pr[:], in_=ppool[:])
                nc.sync.dma_start(out=arin[:], in_=pr[:])
            pp_ctx.__exit__(None, None, None)
            with (
                tc.tile_pool(name="r5", bufs=1) as r5,
                tc.tile_pool(name="r5q", bufs=1, space="PSUM") as r5q,
            ):
                tc.strict_bb_all_engine_barrier()
                nc.gpsimd.collective_compute(
                    "AllReduce", OP.add, replica_groups=rg_all,
                    ins=[arin[:]], outs=[arout[:]])
                tc.strict_bb_all_engine_barrier()
                ar = r5.tile([128, 65], F32)
                nc.sync.dma_start(out=ar[:], in_=arout[:])
                cm = r5.tile([128, 1], F32)
                nc.vector.tensor_scalar(out=cm[:], in0=ar[:, 64:65], scalar1=1.0,
                                        scalar2=None, op0=OP.max)
                cr = r5.tile([128, 1], F32)
                nc.vector.reciprocal(out=cr[:], in_=cm[:])
                gf = r5.tile([128, 64], F32)
                nc.vector.tensor_scalar(out=gf[:], in0=ar[:, 0:64], scalar1=cr[:, 0:1],
                                        scalar2=None, op0=OP.mult)
                identf = r5.tile([128, 128], F32)
                make_identity(nc, identf[:])
                pgt = r5q.tile([64, 128], F32)
                nc.tensor.transpose(out=pgt[:], in_=gf[:], identity=identf[:])
                gfT = r5.tile([64, 128], F32)
                nc.scalar.copy(out=gfT[:], in_=pgt[:])
                plg = r5q.tile([128, 10], F32)
                nc.tensor.matmul(out=plg[:], lhsT=gfT[:], rhs=fcw_s[:], start=True, stop=True)
                lg = r5.tile([128, 16], F32)
                nc.vector.tensor_add(out=lg[:, 0:10], in0=plg[:], in1=fcbb_s[:])
                mx = r5.tile([128, 1], F32)
                nc.vector.reduce_max(out=mx[:], in_=lg[:, 0:10], axis=mybir.AxisListType.X)
                tsh = r5.tile([128, 16], F32)
                nc.vector.tensor_scalar(out=tsh[:, 0:10], in0=lg[:, 0:10],
                                        scalar1=mx[:, 0:1], scalar2=None, op0=OP.subtract)
                exs = r5.tile([128, 16], F32)
                se = r5.tile([128, 1], F32)
                nc.scalar.activation(out=exs[:, 0:10], in_=tsh[:, 0:10], func=AF.Exp,
                                     accum_out=se[:])
                ln = r5.tile([128, 1], F32)
                nc.scalar.activation(out=ln[:], in_=se[:], func=AF.Ln)
                res = r5.tile([128, 16], F32)
                nc.vector.memset(res[:], 0.0)
                nc.vector.tensor_scalar(out=res[:, 0:10], in0=tsh[:, 0:10],
                                        scalar1=ln[:, 0:1], scalar2=None, op0=OP.subtract)
                nc.sync.dma_start(out=out[:], in_=res[:])
    nc.compile()
    return nc


# ---------------- self-contained entry point ----------------
_CACHE = {}


def kernel(**inputs):
    """Full DAGNN forward. Takes the unsharded inputs from setup_inputs();
    returns log-softmax output [num_graphs, 10] float32."""
    x = np.asarray(inputs["x"], np.float32)
    edge_index = np.asarray(inputs["edge_index"])
    batch = np.asarray(inputs["batch"])
    G = int(inputs["num_graphs"])
    weights = [np.asarray(inputs[k], np.float32) for k in (
        "gru_w_ih", "gru_w_hh", "gru_b_ih", "gru_b_hh",
        "W1", "att_src1", "att_dst1", "b1",
        "W2", "att_src2", "att_dst2", "b2", "fc_w", "fc_b")]
    N = x.shape[0]
    E = edge_index.shape[1]
    P = 8

    from concourse.bass_utils import run_bass_kernel_spmd
    cfg = Cfg(N, E, G, P)
    per_core = host_prep(cfg, edge_index, batch)
    in_maps = build_inputs(cfg, x, weights, per_core)
    key = (N, E, G, P, cfg.NPAD2, cfg.TOT_TILES, tuple(cfg.tiles[:8]))
    if key not in _CACHE:
        _CACHE[key] = build_kernel(cfg)
    nc = _CACHE[key]
    res = run_bass_kernel_spmd(nc, in_maps, core_ids=list(range(P)))
    out = np.asarray(res.results[0]["out"][:G, :10], np.float32)
    return out


# revision 19
# speedup vs baseline: 1.0333x; 1.0333x over previous
"""DAGNN (GRU + 2xGAT + mean-pool + fc + log_softmax) on 8 TRN2 cores via Bass/Tile.

Sharding: nodes split across cores by dst-range (edges sorted by dst, split at
dst boundaries), so each core's GRU computes exactly the h/attention values its
GAT dst windows need locally. Edge payload gathers use batched dma_gather from
a 4-way row-sharded bf16 table (int16 index limit); per-window dst attention
terms are expanded on-chip via selection-matrix matmuls. Feature tables are
AllGathered; graph pooling partial sums are AllReduced.
"""
import sys
import numpy as np

sys.path.insert(0, "/opt/trn_rl_repo")

import ml_dtypes
import concourse.bass as bass
import concourse.bacc as bacc
import concourse.mybir as mybir
import concourse.tile as tile
from concourse.masks import make_identity

F32 = mybir.dt.float32
BF16 = mybir.dt.bfloat16
I16 = mybir.dt.int16
U8 = mybir.dt.uint8
AF = mybir.ActivationFunctionType
OP = mybir.AluOpType

NSH = 4          # table row shards (int16 gather index limit)
GT = 512         # GRU node tile


def _ceil(a, b):
    return -(-a // b)


class Cfg:
    def __init__(self, N, E, G, P):
        self.N, self.E, self.G, self.P = N, E, G, P
        self.T, self.D, self.H = 8, 128, 64
        self.HEADS, self.C1, self.C2 = 4, 256, 64


def host_prep(cfg, edge_index, batch):
    N, E, P = cfg.N, cfg.E, cfg.P
    src = np.concatenate([np.asarray(edge_index[0], np.int64), np.arange(N, dtype=np.int64)])
    dst = np.concatenate([np.asarray(edge_index[1], np.int64), np.arange(N, dtype=np.int64)])
    order = np.argsort(dst, kind="stable")
    ss, dd = src[order], dst[order]
    Etot = ss.shape[0]

    bounds = [0]
    for k in range(1, P):
        pos = (k * Etot) // P
        while pos < Etot and dd[pos] == dd[pos - 1]:
            pos += 1
        bounds.append(pos)
    bounds.append(Etot)
    n0 = np.zeros(P + 1, np.int64)
    n0[P] = N
    for c in range(1, P):
        n0[c] = dd[bounds[c]]
    ranges = np.diff(n0)
    NPAD2 = _ceil(int(ranges.max()), GT) * GT
    NW = NPAD2 // 128
    SH = (P * NPAD2) // NSH
    assert SH - 1 <= 32767, f"shard too large for int16: {SH}"
    cfg.n0, cfg.NPAD2, cfg.NW, cfg.SH = n0, NPAD2, NW, SH
    cfg.NT = NPAD2 // GT

    owner = np.searchsorted(n0[1:P], np.arange(N), side="right")
    g2r = owner * NPAD2 + (np.arange(N) - n0[owner])
    shard_of = (g2r // SH).astype(np.int64)
    rel_of = (g2r % SH).astype(np.int16)

    # pass 1: per-(window, shard) edge counts per core -> uniform tile counts
    NB = NW * NSH
    kws = np.zeros((P, NB), np.int64)
    per_edges = []
    for c in range(P):
        sl = slice(bounds[c], bounds[c + 1])
        ssc, ddc = ss[sl], dd[sl]
        w_arr = (ddc - n0[c]) // 128
        s_arr = shard_of[ssc]
        key = (w_arr * NSH + s_arr).astype(np.int64)
        kws[c] = np.bincount(key, minlength=NB)
        per_edges.append((ssc, ddc, w_arr, key))
    tiles = np.maximum(1, _ceil(kws.max(axis=0), 128)).astype(np.int64)
    tile_off = np.concatenate([[0], np.cumsum(tiles)])
    TOT_TILES = int(tile_off[-1])
    cfg.tiles, cfg.tile_off, cfg.TOT_TILES = tiles, tile_off, TOT_TILES
    cfg.TBMAX = int(tiles.max())

    per_core = []
    for c in range(P):
        ssc, ddc, w_arr, key = per_edges[c]
        order2 = np.argsort(key, kind="stable")
        sk = key[order2]
        grp_start = np.searchsorted(sk, np.arange(NB))
        rank = np.arange(sk.shape[0]) - grp_start[sk]
        slotpos = tile_off[sk] * 128 + rank
        TOT_SLOT = TOT_TILES * 128
        srel = np.zeros(TOT_SLOT, np.int16)
        drel = np.full(TOT_SLOT, 255, np.uint8)
        srel[slotpos] = rel_of[ssc[order2]]
        drel[slotpos] = (ddc[order2] - n0[c] - 128 * w_arr[order2]).astype(np.uint8)
        # wrapped gather indices: idx i of a (tile-aligned) run at [i%16, i//16]
        wr = np.ascontiguousarray(srel.reshape(TOT_SLOT // 16, 16).T)
        idx_wr = np.tile(wr, (8, 1))                                   # [128, TOT_SLOT//16]
        drel_pt = np.ascontiguousarray(drel.reshape(TOT_TILES, 128).T)  # [128, TOT_TILES]
        drelT = drel.reshape(1, TOT_TILES, 128).copy()                  # [1, TOT_TILES, 128]
        bd = np.full(NPAD2, 999.0, np.float32)
        rg = int(ranges[c])
        bd[:rg] = np.asarray(batch, np.int64)[n0[c]:n0[c + 1]].astype(np.float32)
        batch_wd = np.ascontiguousarray(bd.reshape(NW, 128).T)          # [128, NW]
        per_core.append(dict(idx_wr=idx_wr, drel_pt=drel_pt, drelT=drelT,
                             batch_wd=batch_wd, rg=rg))
    return per_core


def build_inputs(cfg, x, weights, per_core):
    (gru_w_ih, gru_w_hh, gru_b_ih, gru_b_hh, W1, att_src1, att_dst1, b1,
     W2, att_src2, att_dst2, b2, fc_w, fc_b) = weights
    P, NPAD2 = cfg.P, cfg.NPAD2
    bf = ml_dtypes.bfloat16

    # BC1 [64, 8]: cols 0:4 src-att coeffs per head, 4:8 dst-att
    BC1 = np.zeros((64, 8), np.float32)
    for h in range(4):
        Wh = W1[:, 64 * h:64 * (h + 1)]
        BC1[:, h] = Wh @ att_src1[h]
        BC1[:, 4 + h] = Wh @ att_dst1[h]
    # W1 block-diagonal pairs for transposed apply
    W12A = np.zeros((128, 128), np.float32)
    W12B = np.zeros((128, 128), np.float32)
    W12A[0:64, 0:64] = W1[:, 0:64]
    W12A[64:128, 64:128] = W1[:, 64:128]
    W12B[0:64, 0:64] = W1[:, 128:192]
    W12B[64:128, 64:128] = W1[:, 192:256]
    # W2 halves side by side; BC2 [128, 4]: cols 2h = [src|dst] coeffs, half h
    W2s = np.zeros((128, 128), np.float32)
    W2s[:, 0:64] = W2[0:128, :]
    W2s[:, 64:128] = W2[128:256, :]
    a2 = W2 @ att_src2[0]   # [256]
    d2 = W2 @ att_dst2[0]
    BC2 = np.zeros((128, 4), np.float32)
    BC2[:, 0] = a2[0:128]
    BC2[:, 1] = d2[0:128]
    BC2[:, 2] = a2[128:256]
    BC2[:, 3] = d2[128:256]

    com = dict(
        wihT=np.ascontiguousarray(gru_w_ih.T).astype(bf),               # [128,192]
        whrz=np.concatenate([gru_w_hh.T[:, 0:128],
                             (gru_b_ih + gru_b_hh)[None, 0:128]], 0).astype(bf),  # [65,128]
        whn=np.concatenate([gru_w_hh.T[:, 128:192],
                            gru_b_hh[None, 128:192]], 0).astype(bf),    # [65,64]
        bihn=np.ascontiguousarray(gru_b_ih[128:192].reshape(64, 1)).astype(np.float32),
        BC1=BC1.astype(bf),
        W12A=W12A.astype(bf), W12B=W12B.astype(bf),
        b1b=np.broadcast_to(b1, (128, 256)).astype(bf).copy(),
        W2s=W2s.astype(bf), BC2=BC2.astype(bf),
        b2b=np.broadcast_to(b2, (128, 64)).astype(bf).copy(),
        fcw=fc_w.astype(np.float32),
        fcbb=np.broadcast_to(fc_b, (128, 10)).astype(np.float32).copy(),
    )
    in_maps = []
    for c in range(P):
        pc = per_core[c]
        rg = pc["rg"]
        xp = np.zeros((NPAD2, cfg.T, cfg.D), np.float32)
        xp[:rg] = x[cfg.n0[c]:cfg.n0[c + 1]]
        xpT = np.ascontiguousarray(xp.transpose(1, 2, 0)).astype(bf)     # [8,128,NPAD2]
        m = dict(com)
        m.update(xpT=xpT, idx_wr=pc["idx_wr"], drel_pt=pc["drel_pt"],
                 drelT=pc["drelT"], batch_wd=pc["batch_wd"])
        in_maps.append(m)
    return in_maps


def build_kernel(cfg, dbg=False):
    P, T, NPAD2, NW, SH = cfg.P, cfg.T, cfg.NPAD2, cfg.NW, cfg.SH
    NT, NSHARD = cfg.NT, NSH
    tiles, tile_off, TOT_TILES = cfg.tiles, cfg.tile_off, cfg.TOT_TILES
    TOT_SLOT = TOT_TILES * 128
    rg_all = [list(range(P))]

    nc = bacc.Bacc("TRN2", target_bir_lowering=False, debug=False,
                   dynamic_dma_scratch_size=32768)
    # inputs
    xpT = nc.dram_tensor("xpT", [T, 128, NPAD2], BF16, kind="ExternalInput")
    wihT = nc.dram_tensor("wihT", [128, 192], BF16, kind="ExternalInput")
    whrz = nc.dram_tensor("whrz", [65, 128], BF16, kind="ExternalInput")
    whn = nc.dram_tensor("whn", [65, 64], BF16, kind="ExternalInput")
    bihn = nc.dram_tensor("bihn", [64, 1], F32, kind="ExternalInput")
    BC1 = nc.dram_tensor("BC1", [64, 8], BF16, kind="ExternalInput")
    W12A = nc.dram_tensor("W12A", [128, 128], BF16, kind="ExternalInput")
    W12B = nc.dram_tensor("W12B", [128, 128], BF16, kind="ExternalInput")
    b1b = nc.dram_tensor("b1b", [128, 256], BF16, kind="ExternalInput")
    W2s = nc.dram_tensor("W2s", [128, 128], BF16, kind="ExternalInput")
    BC2 = nc.dram_tensor("BC2", [128, 4], BF16, kind="ExternalInput")
    b2b = nc.dram_tensor("b2b", [128, 64], BF16, kind="ExternalInput")
    fcw = nc.dram_tensor("fcw", [64, 10], F32, kind="ExternalInput")
    fcbb = nc.dram_tensor("fcbb", [128, 10], F32, kind="ExternalInput")
    idx_wr = nc.dram_tensor("idx_wr", [128, TOT_SLOT // 16], I16, kind="ExternalInput")
    drel_pt = nc.dram_tensor("drel_pt", [128, TOT_TILES], U8, kind="ExternalInput")
    drelT = nc.dram_tensor("drelT", [1, TOT_TILES, 128], U8, kind="ExternalInput")
    batch_wd = nc.dram_tensor("batch_wd", [128, NW], F32, kind="ExternalInput")
    out = nc.dram_tensor("out", [128, 16], F32, kind="ExternalOutput")
    # internal dram
    xcat1l = nc.dram_tensor("xcat1l", [NPAD2, 128], BF16)
    table1 = nc.dram_tensor("table1", [P * NPAD2, 128], BF16, addr_space="Shared")
    tb1s = [nc.dram_tensor(f"tb1s{s}", [SH, 128], BF16) for s in range(NSHARD)]
    xcat2l = nc.dram_tensor("xcat2l", [NPAD2, 128], BF16)
    table2 = nc.dram_tensor("table2", [P * NPAD2, 128], BF16, addr_space="Shared")
    tb2s = [nc.dram_tensor(f"tb2s{s}", [SH, 128], BF16) for s in range(NSHARD)]
    arin = nc.dram_tensor("arin", [128, 65], F32)
    arout = nc.dram_tensor("arout", [128, 65], F32, addr_space="Shared")

    with tile.TileContext(nc) as tc:
        with tc.tile_pool(name="pers", bufs=1) as pers:
            # ---- persistent: weights, indices, iotas ----
            identb = pers.tile([128, 128], BF16)
            make_identity(nc, identb[:])
            iota_row = pers.tile([128, 128], U8)
            nc.gpsimd.iota(iota_row[:], pattern=[[1, 128]], base=0, channel_multiplier=0,
                           allow_small_or_imprecise_dtypes=True)
            iota_p = pers.tile([128, 1], F32)
            nc.gpsimd.iota(iota_p[:], pattern=[[0, 1]], base=0, channel_multiplier=1,
                           allow_small_or_imprecise_dtypes=True)
            wihT_s = pers.tile([128, 192], BF16)
            nc.sync.dma_start(out=wihT_s[:], in_=wihT[:])
            whrz_s = pers.tile([65, 128], BF16)
            nc.sync.dma_start(out=whrz_s[:], in_=whrz[:])
            whn_s = pers.tile([65, 64], BF16)
            nc.sync.dma_start(out=whn_s[:], in_=whn[:])
            bihn_s = pers.tile([64, 1], F32)
            nc.sync.dma_start(out=bihn_s[:], in_=bihn[:])
            BC1_s = pers.tile([64, 8], BF16)
            nc.sync.dma_start(out=BC1_s[:], in_=BC1[:])
            W12A_s = pers.tile([128, 128], BF16)
            nc.sync.dma_start(out=W12A_s[:], in_=W12A[:])
            W12B_s = pers.tile([128, 128], BF16)
            nc.sync.dma_start(out=W12B_s[:], in_=W12B[:])
            b1b_s = pers.tile([128, 256], BF16)
            nc.sync.dma_start(out=b1b_s[:], in_=b1b[:])
            W2s_s = pers.tile([128, 128], BF16)
            nc.sync.dma_start(out=W2s_s[:], in_=W2s[:])
            BC2_s = pers.tile([128, 4], BF16)
            nc.sync.dma_start(out=BC2_s[:], in_=BC2[:])
            b2b_s = pers.tile([128, 64], BF16)
            nc.sync.dma_start(out=b2b_s[:], in_=b2b[:])
            fcw_s = pers.tile([64, 10], F32)
            nc.sync.dma_start(out=fcw_s[:], in_=fcw[:])
            fcbb_s = pers.tile([128, 10], F32)
            nc.sync.dma_start(out=fcbb_s[:], in_=fcbb[:])
            idx_sb = pers.tile([128, TOT_SLOT // 16], I16)
            nc.sync.dma_start(out=idx_sb[:], in_=idx_wr[:])
            drel_sb = pers.tile([128, TOT_TILES], U8)
            nc.sync.dma_start(out=drel_sb[:], in_=drel_pt[:])
            batch_sb = pers.tile([128, NW], F32)
            nc.sync.dma_start(out=batch_sb[:], in_=batch_wd[:])
            adt1_sb = pers.tile([128, NW, 4], BF16)
            adt2_sb = pers.tile([128, NW, 1], BF16)
            nidx_regs = {}
            for b in range(NW * NSHARD):
                Tb = int(tiles[b])
                for q0 in range(0, Tb, 8):
                    n = min(8, Tb - q0) * 128
                    if n not in nidx_regs:
                        nidx_regs[n] = nc.gpsimd.to_reg(n)

            # ---- phase 1: GRU -> xcat1l (h|asrc), adt1_sb ----
            with (
                tc.tile_pool(name="gx", bufs=2) as gx,
                tc.tile_pool(name="gh", bufs=2) as gh,
                tc.tile_pool(name="gv", bufs=3) as gv,
                tc.tile_pool(name="gp1", bufs=1, space="PSUM") as gp1,
                tc.tile_pool(name="gp2", bufs=1, space="PSUM") as gp2,
                tc.tile_pool(name="gp3", bufs=1, space="PSUM") as gp3,
                tc.tile_pool(name="gp4", bufs=2, space="PSUM") as gp4,
            ):
                for it in range(NT):
                    xt8 = gx.tile([128, T, GT], BF16, tag="xt8")
                    for t in range(T):
                        nc.sync.dma_start(out=xt8[:, t, :],
                                          in_=xpT[t, :, it * GT:(it + 1) * GT])
                    hT = gh.tile([65, GT], BF16, tag="hT")
                    nc.vector.memset(hT[0:64, :], 0.0)
                    nc.vector.memset(hT[64:65, :], 1.0)
                    for t in range(T):
                        prz = gp1.tile([64, 2 * GT], F32, tag="prz")
                        nc.tensor.matmul(out=prz[:, 0:GT], lhsT=wihT_s[:, 0:64],
                                         rhs=xt8[:, t, :], start=True, stop=False)
                        nc.tensor.matmul(out=prz[:, 0:GT], lhsT=whrz_s[:, 0:64],
                                         rhs=hT[:], start=False, stop=True)
                        nc.tensor.matmul(out=prz[:, GT:2 * GT], lhsT=wihT_s[:, 64:128],
                                         rhs=xt8[:, t, :], start=True, stop=False)
                        nc.tensor.matmul(out=prz[:, GT:2 * GT], lhsT=whrz_s[:, 64:128],
                                         rhs=hT[:], start=False, stop=True)
                        pin = gp2.tile([64, GT], F32, tag="pin")
                        nc.tensor.matmul(out=pin[:], lhsT=wihT_s[:, 128:192],
                                         rhs=xt8[:, t, :], start=True, stop=True)
                        phn = gp3.tile([64, GT], F32, tag="phn")
                        nc.tensor.matmul(out=phn[:], lhsT=whn_s[:], rhs=hT[:],
                                         start=True, stop=True)
                        rz = gv.tile([64, 2 * GT], BF16, tag="rz")
                        nc.scalar.activation(out=rz[:], in_=prz[:], func=AF.Sigmoid)
                        tmp = gv.tile([64, GT], BF16, tag="tmp")
                        nc.vector.tensor_mul(out=tmp[:], in0=rz[:, 0:GT], in1=phn[:])
                        t3 = gv.tile([64, GT], F32, tag="t3")
                        nc.vector.tensor_add(out=t3[:], in0=pin[:], in1=tmp[:])
                        nh = gv.tile([64, GT], BF16, tag="nh")
                        nc.scalar.activation(out=nh[:], in_=t3[:], func=AF.Tanh,
                                             bias=bihn_s[:, 0:1])
                        s1 = gv.tile([64, GT], BF16, tag="s1")
                        nc.vector.tensor_sub(out=s1[:], in0=hT[0:64, :], in1=nh[:])
                        s2 = gv.tile([64, GT], BF16, tag="s2")
                        nc.vector.tensor_mul(out=s2[:], in0=rz[:, GT:2 * GT], in1=s1[:])
                        nc.vector.tensor_add(out=hT[0:64, :], in0=nh[:], in1=s2[:])
                    for cc in range(GT // 128):
                        w = it * (GT // 128) + cc
                        pt = gp4.tile([128, 64], BF16, tag="pt")
                        nc.tensor.transpose(out=pt[:],
                                            in_=hT[0:64, cc * 128:(cc + 1) * 128],
                                            identity=identb[0:64, 0:64])
                        pat = gp4.tile([128, 8], F32, tag="pat")
                        nc.tensor.matmul(out=pat[:], lhsT=hT[0:64, cc * 128:(cc + 1) * 128],
                                         rhs=BC1_s[:], start=True, stop=True)
                        xc = gv.tile([128, 128], BF16, tag="xc")
                        nc.scalar.copy(out=xc[:, 0:64], in_=pt[:])
                        nc.vector.tensor_copy(out=xc[:, 64:68], in_=pat[:, 0:4])
                        nc.vector.memset(xc[:, 68:128], 0.0)
                        nc.vector.tensor_copy(out=adt1_sb[:, w, :], in_=pat[:, 4:8])
                        nc.sync.dma_start(out=xcat1l[w * 128:(w + 1) * 128, :], in_=xc[:])

            # ---- AllGather table1, split into shards ----
            nc.gpsimd.collective_compute(
                "AllGather", OP.bypass, replica_groups=rg_all,
                ins=[xcat1l[:]], outs=[table1[:]])
            for s in range(NSHARD):
                for ci in range(2):
                    nc.sync.dma_start(
                        out=tb1s[s][ci * NPAD2:(ci + 1) * NPAD2, :],
                        in_=table1[(2 * s + ci) * NPAD2:(2 * s + ci + 1) * NPAD2, :])

            # ---- GAT layer over windows (shared for layer 1 / layer 2) ----
            def gat_windows(tbls, adt_sb, nheads, payw, post_fn):
                """payw: scatter matmul width (4+256 for L1, 1+64 for L2)."""
                with (
                    tc.tile_pool(name="pg", bufs=3) as pg,
                    tc.tile_pool(name="pS", bufs=2) as pS,
                    tc.tile_pool(name="pd", bufs=2) as pd,
                    tc.tile_pool(name="pu", bufs=2) as pu,
                    tc.tile_pool(name="pM", bufs=2) as pM,
                    tc.tile_pool(name="pw", bufs=2) as pw,
                    tc.tile_pool(name="ps2", bufs=2, space="PSUM") as ps2,
                    tc.tile_pool(name="psA", bufs=1, space="PSUM") as psA,
                    tc.tile_pool(name="psB", bufs=1, space="PSUM") as psB,
                ):
                    for w in range(NW):
                        pblk = ps2.tile([128, payw], F32, tag="pblk")
                        first = True
                        for s in range(NSHARD):
                            b = w * NSHARD + s
                            Tb = int(tiles[b])
                            t0 = int(tile_off[b])
                            g = pg.tile([128, Tb, 128], BF16, tag=f"g{Tb}")
                            for q0 in range(0, Tb, 8):
                                qn = min(8, Tb - q0)
                                nc.gpsimd.dma_gather(
                                    out_ap=g[:, q0:q0 + qn, :], in_ap=tbls[s][:],
                                    idxs_ap=idx_sb[:, (t0 + q0) * 8:(t0 + q0 + qn) * 8],
                                    num_idxs=qn * 128, num_idxs_reg=nidx_regs[qn * 128],
                                    elem_size=128)
                            S = pS.tile([128, Tb, 128], BF16, tag=f"S{Tb}")
                            nc.vector.tensor_tensor(
                                out=S[:],
                                in0=iota_row[:].unsqueeze(1).broadcast_to([128, Tb, 128]),
                                in1=drel_sb[:, t0:t0 + Tb].unsqueeze(2).broadcast_to([128, Tb, 128]),
                                op=OP.is_equal)
                            drT = pd.tile([128, Tb, 128], U8, tag=f"dT{Tb}")
                            nc.sync.dma_start(
                                out=drT[:],
                                in_=drelT[0:1, t0:t0 + Tb, :].partition_broadcast(128))
                            Sd = pS.tile([128, Tb, 128], BF16, tag=f"Sd{Tb}")
                            nc.vector.tensor_scalar(
                                out=Sd[:], in0=drT[:], scalar1=iota_p[:, 0:1],
                                scalar2=None, op0=OP.is_equal)
                            padp = psA.tile([128, Tb * nheads], F32, tag="padp")
                            for t in range(Tb):
                                nc.tensor.matmul(
                                    out=padp[:, t * nheads:(t + 1) * nheads],
                                    lhsT=Sd[:, t, :], rhs=adt_sb[:, w, :],
                                    start=True, stop=True)
                            u = pu.tile([128, Tb, nheads], F32, tag=f"u{Tb}")
                            nc.vector.tensor_add(
                                out=u[:], in0=g[:, :, 64:64 + nheads],
                                in1=padp[:].rearrange("p (t c) -> p t c", t=Tb))
                            e1 = pu.tile([128, Tb, nheads], BF16, tag=f"e1{Tb}")
                            nc.scalar.activation(out=e1[:], in_=u[:], func=AF.Exp)
                            e2 = pu.tile([128, Tb, nheads], BF16, tag=f"e2{Tb}")
                            nc.scalar.activation(out=e2[:], in_=u[:], func=AF.Exp, scale=0.2)
                            ee = pu.tile([128, Tb, nheads], BF16, tag=f"ee{Tb}")
                            nc.vector.tensor_tensor(out=ee[:], in0=e1[:], in1=e2[:], op=OP.max)
                            M = pM.tile([128, Tb, payw], BF16, tag=f"M{Tb}")
                            nc.vector.tensor_copy(out=M[:, :, 0:nheads], in_=ee[:])
                            if nheads == 4:
                                nc.vector.tensor_tensor(
                                    out=M[:, :, 4:260].rearrange("p t (h c) -> p t h c", h=4),
                                    in0=g[:, :, 0:64].unsqueeze(2).broadcast_to([128, Tb, 4, 64]),
                                    in1=ee[:].unsqueeze(3).broadcast_to([128, Tb, 4, 64]),
                                    op=OP.mult)
                            else:
                                nc.vector.tensor_tensor(
                                    out=M[:, :, 1:65],
                                    in0=g[:, :, 0:64],
                                    in1=ee[:].to_broadcast([128, Tb, 64]),
                                    op=OP.mult)
                            for t in range(Tb):
                                nc.tensor.matmul(
                                    out=pblk[:], lhsT=S[:, t, :], rhs=M[:, t, :],
                                    start=first, stop=(s == NSHARD - 1 and t == Tb - 1))
                                first = False
                        post_fn(w, pblk, pw, psB)

            # ---- layer 1 post: h1 = relu(z/denom @ W1 + b1) -> xcat2, adt2 ----
            def post1(w, pblk, pw, psB):
                dn = pw.tile([128, 4], F32, tag="dn")
                nc.vector.tensor_scalar(out=dn[:], in0=pblk[:, 0:4], scalar1=1e-12,
                                        scalar2=None, op0=OP.max)
                rec = pw.tile([128, 4], F32, tag="rec")
                nc.vector.reciprocal(out=rec[:], in_=dn[:])
                zn = pw.tile([128, 256], BF16, tag="zn")
                nc.vector.tensor_tensor(
                    out=zn[:].rearrange("p (h c) -> p h c", h=4),
                    in0=pblk[:, 4:260].rearrange("p (h c) -> p h c", h=4),
                    in1=rec[:].unsqueeze(2).broadcast_to([128, 4, 64]),
                    op=OP.mult)
                tpz = pw.tile([128, 256], BF16, tag="tpz")
                for half in range(2):
                    ptp = psB.tile([128, 128], BF16, tag="ptp")
                    nc.tensor.transpose(out=ptp[:], in_=zn[:, 128 * half:128 * (half + 1)],
                                        identity=identb[:])
                    nc.scalar.copy(out=tpz[:, 128 * half:128 * (half + 1)], in_=ptp[:])
                h1p = psB.tile([128, 256], F32, tag="h1p")
                nc.tensor.matmul(out=h1p[:, 0:128], lhsT=tpz[:, 0:128], rhs=W12A_s[:],
                                 start=True, stop=True)
                nc.tensor.matmul(out=h1p[:, 128:256], lhsT=tpz[:, 128:256], rhs=W12B_s[:],
                                 start=True, stop=True)
                h1 = pw.tile([128, 256], BF16, tag="h1")
                nc.vector.tensor_add(out=h1[:], in0=h1p[:], in1=b1b_s[:])
                nc.vector.tensor_scalar(out=h1[:], in0=h1[:], scalar1=0.0,
                                        scalar2=None, op0=OP.max)
                th1 = pw.tile([128, 256], BF16, tag="th1")
                for half in range(2):
                    ptp = psB.tile([128, 128], BF16, tag="ptp")
                    nc.tensor.transpose(out=ptp[:], in_=h1[:, 128 * half:128 * (half + 1)],
                                        identity=identb[:])
                    nc.scalar.copy(out=th1[:, 128 * half:128 * (half + 1)], in_=ptp[:])
                xsc = psB.tile([128, 66], F32, tag="xsc")
                xs2p = xsc[:, 0:64]
                pat2 = xsc[:, 64:66]
                for half in range(2):
                    nc.tensor.matmul(out=xs2p, lhsT=th1[:, 128 * half:128 * (half + 1)],
                                     rhs=W2s_s[:, 64 * half:64 * (half + 1)],
                                     start=(half == 0), stop=(half == 1))
                    nc.tensor.matmul(out=pat2, lhsT=th1[:, 128 * half:128 * (half + 1)],
                                     rhs=BC2_s[:, 2 * half:2 * half + 2],
                                     start=(half == 0), stop=(half == 1))
                xc2 = pw.tile([128, 128], BF16, tag="xc2")
                nc.scalar.copy(out=xc2[:, 0:64], in_=xs2p)
                nc.vector.tensor_copy(out=xc2[:, 64:65], in_=pat2[:, 0:1])
                nc.vector.memset(xc2[:, 65:128], 0.0)
                nc.vector.tensor_copy(out=adt2_sb[:, w, :], in_=pat2[:, 1:2])
                nc.sync.dma_start(out=xcat2l[w * 128:(w + 1) * 128, :], in_=xc2[:])

            gat_windows(tb1s, adt1_sb, 4, 260, post1)

            nc.gpsimd.collective_compute(
                "AllGather", OP.bypass, replica_groups=rg_all,
                ins=[xcat2l[:]], outs=[table2[:]])
            for s in range(NSHARD):
                for ci in range(2):
                    nc.sync.dma_start(
                        out=tb2s[s][ci * NPAD2:(ci + 1) * NPAD2, :],
                        in_=table2[(2 * s + ci) * NPAD2:(2 * s + ci + 1) * NPAD2, :])

            # ---- layer 2 post: pooling into ppool psum ----
            pp_ctx = tc.tile_pool(name="psPool", bufs=1, space="PSUM")
            psPool = pp_ctx.__enter__()
            ppool = psPool.tile([128, 65], F32)

            def post2(w, pblk, pw, psB):
                dn2 = pw.tile([128, 1], F32, tag="dn2")
                nc.vector.tensor_scalar(out=dn2[:], in0=pblk[:, 0:1], scalar1=1e-12,
                                        scalar2=None, op0=OP.max)
                rec2 = pw.tile([128, 1], F32, tag="rec2")
                nc.vector.reciprocal(out=rec2[:], in_=dn2[:])
                ph = pw.tile([128, 65], BF16, tag="ph")
                nc.vector.tensor_scalar(out=ph[:, 0:64], in0=pblk[:, 1:65],
                                        scalar1=rec2[:, 0:1], scalar2=None, op0=OP.mult)
                nc.vector.tensor_add(out=ph[:, 0:64], in0=ph[:, 0:64], in1=b2b_s[:])
                nc.vector.tensor_scalar(out=ph[:, 0:64], in0=ph[:, 0:64], scalar1=0.0,
                                        scalar2=None, op0=OP.max)
                nc.vector.memset(ph[:, 64:65], 1.0)
                Sb = pw.tile([128, 128], BF16, tag="Sb")
                nc.vector.tensor_scalar(out=Sb[:], in0=iota_row[:],
                                        scalar1=batch_sb[:, w:w + 1], scalar2=None,
                                        op0=OP.is_equal)
                nc.tensor.matmul(out=ppool[:], lhsT=Sb[:], rhs=ph[:],
                                 start=(w == 0), stop=(w == NW - 1))

            gat_windows(tb2s, adt2_sb, 1, 65, post2)

            # ---- tail: AllReduce pools, fc, log_softmax ----
            with tc.tile_pool(name="rpre", bufs=1) as rpre:
                pr = rpre.tile([128, 65], F32)
                nc.scalar.copy(out=pr[:], in_=ppool[:])
                nc.sync.dma_start(out=arin[:], in_=pr[:])
            pp_ctx.__exit__(None, None, None)
            with (
                tc.tile_pool(name="r5", bufs=1) as r5,
                tc.tile_pool(name="r5q", bufs=1, space="PSUM") as r5q,
            ):
                tc.strict_bb_all_engine_barrier()
                nc.gpsimd.collective_compute(
                    "AllReduce", OP.add, replica_groups=rg_all,
                    ins=[arin[:]], outs=[arout[:]])
                tc.strict_bb_all_engine_barrier()
                ar = r5.tile([128, 65], F32)
                nc.sync.dma_start(out=ar[:], in_=arout[:])
                cm = r5.tile([128, 1], F32)
                nc.vector.tensor_scalar(out=cm[:], in0=ar[:, 64:65], scalar1=1.0,
                                        scalar2=None, op0=OP.max)
                cr = r5.tile([128, 1], F32)
                nc.vector.reciprocal(out=cr[:], in_=cm[:])
                gf = r5.tile([128, 64], F32)
                nc.vector.tensor_scalar(out=gf[:], in0=ar[:, 0:64], scalar1=cr[:, 0:1],
                                        scalar2=None, op0=OP.mult)
                identf = r5.tile([128, 128], F32)
                make_identity(nc, identf[:])
                pgt = r5q.tile([64, 128], F32)
                nc.tensor.transpose(out=pgt[:], in_=gf[:], identity=identf[:])
                gfT = r5.tile([64, 128], F32)
                nc.scalar.copy(out=gfT[:], in_=pgt[:])
                plg = r5q.tile([128, 10], F32)
                nc.tensor.matmul(out=plg[:], lhsT=gfT[:], rhs=fcw_s[:], start=True, stop=True)
                lg = r5.tile([128, 16], F32)
                nc.vector.tensor_add(out=lg[:, 0:10], in0=plg[:], in1=fcbb_s[:])
                mx = r5.tile([128, 1], F32)
                nc.vector.reduce_max(out=mx[:], in_=lg[:, 0:10], axis=mybir.AxisListType.X)
                tsh = r5.tile([128, 16], F32)
                nc.vector.tensor_scalar(out=tsh[:, 0:10], in0=lg[:, 0:10],
                                        scalar1=mx[:, 0:1], scalar2=None, op0=OP.subtract)
                exs = r5.tile([128, 16], F32)
                se = r5.tile([128, 1], F32)
                nc.scalar.activation(out=exs[:, 0:10], in_=tsh[:, 0:10], func=AF.Exp,
                                     accum_out=se[:])
                ln = r5.tile([128, 1], F32)
                nc.scalar.activation(out=ln[:], in_=se[:], func=AF.Ln)
                res = r5.tile([128, 16], F32)
                nc.vector.memset(res[:], 0.0)
                nc.vector.tensor_scalar(out=res[:, 0:10], in0=tsh[:, 0:10],
                                        scalar1=ln[:, 0:1], scalar2=None, op0=OP.subtract)
                nc.sync.dma_start(out=out[:], in_=res[:])
    nc.compile()
    return nc


# ---------------- self-contained entry point ----------------
_CACHE = {}


def kernel(**inputs):
    """Full DAGNN forward. Takes the unsharded inputs from setup_inputs();
    returns log-softmax output [num_graphs, 10] float32."""
    x = np.asarray(inputs["x"], np.float32)
    edge_index = np.asarray(inputs["edge_index"])
    batch = np.asarray(inputs["batch"])
    G = int(inputs["num_graphs"])
    weights = [np.asarray(inputs[k], np.float32) for k in (
        "gru_w_ih", "gru_w_hh", "gru_b_ih", "gru_b_hh",
        "W1", "att_src1", "att_dst1", "b1",
        "W2", "att_src2", "att_dst2", "b2", "fc_w", "fc_b")]
    N = x.shape[0]
    E = edge_index.shape[1]
    P = 8

    from concourse.bass_utils import run_bass_kernel_spmd
    cfg = Cfg(N, E, G, P)
    per_core = host_prep(cfg, edge_index, batch)
    in_maps = build_inputs(cfg, x, weights, per_core)
    key = (N, E, G, P, cfg.NPAD2, cfg.TOT_TILES, tuple(cfg.tiles[:8]))
    if key not in _CACHE:
        _CACHE[key] = build_kernel(cfg)
    nc = _CACHE[key]
    res = run_bass_kernel_spmd(nc, in_maps, core_ids=list(range(P)))
    out = np.asarray(res.results[0]["out"][:G, :10], np.float32)
    return out


# revision 20
# speedup vs baseline: 1.0482x; 1.0145x over previous
"""DAGNN (GRU + 2xGAT + mean-pool + fc + log_softmax) on 8 TRN2 cores via Bass/Tile.

Sharding: nodes split across cores by dst-range (edges sorted by dst, split at
dst boundaries), so each core's GRU computes exactly the h/attention values its
GAT dst windows need locally. Edge payload gathers use batched dma_gather from
a 4-way row-sharded bf16 table (int16 index limit); per-window dst attention
terms are expanded on-chip via selection-matrix matmuls. Feature tables are
AllGathered; graph pooling partial sums are AllReduced.
"""
import sys
import numpy as np

sys.path.insert(0, "/opt/trn_rl_repo")

import ml_dtypes
import concourse.bass as bass
import concourse.bacc as bacc
import concourse.mybir as mybir
import concourse.tile as tile
from concourse.masks import make_identity

F32 = mybir.dt.float32
BF16 = mybir.dt.bfloat16
I16 = mybir.dt.int16
U8 = mybir.dt.uint8
AF = mybir.ActivationFunctionType
OP = mybir.AluOpType

NSH = 4          # table row shards (int16 gather index limit)
GT = 512         # GRU node tile


def _ceil(a, b):
    return -(-a // b)


class Cfg:
    def __init__(self, N, E, G, P):
        self.N, self.E, self.G, self.P = N, E, G, P
        self.T, self.D, self.H = 8, 128, 64
        self.HEADS, self.C1, self.C2 = 4, 256, 64


def host_prep(cfg, edge_index, batch):
    N, E, P = cfg.N, cfg.E, cfg.P
    src = np.concatenate([np.asarray(edge_index[0], np.int64), np.arange(N, dtype=np.int64)])
    dst = np.concatenate([np.asarray(edge_index[1], np.int64), np.arange(N, dtype=np.int64)])
    order = np.argsort(dst, kind="stable")
    ss, dd = src[order], dst[order]
    Etot = ss.shape[0]

    bounds = [0]
    for k in range(1, P):
        pos = (k * Etot) // P
        while pos < Etot and dd[pos] == dd[pos - 1]:
            pos += 1
        bounds.append(pos)
    bounds.append(Etot)
    n0 = np.zeros(P + 1, np.int64)
    n0[P] = N
    for c in range(1, P):
        n0[c] = dd[bounds[c]]
    ranges = np.diff(n0)
    NPAD2 = _ceil(int(ranges.max()), GT) * GT
    NW = NPAD2 // 128
    SH = (P * NPAD2) // NSH
    assert SH - 1 <= 32767, f"shard too large for int16: {SH}"
    cfg.n0, cfg.NPAD2, cfg.NW, cfg.SH = n0, NPAD2, NW, SH
    cfg.NT = NPAD2 // GT

    owner = np.searchsorted(n0[1:P], np.arange(N), side="right")
    g2r = owner * NPAD2 + (np.arange(N) - n0[owner])
    shard_of = (g2r // SH).astype(np.int64)
    rel_of = (g2r % SH).astype(np.int16)

    # pass 1: per-(window, shard) edge counts per core -> uniform tile counts
    NB = NW * NSH
    kws = np.zeros((P, NB), np.int64)
    per_edges = []
    for c in range(P):
        sl = slice(bounds[c], bounds[c + 1])
        ssc, ddc = ss[sl], dd[sl]
        w_arr = (ddc - n0[c]) // 128
        s_arr = shard_of[ssc]
        key = (w_arr * NSH + s_arr).astype(np.int64)
        kws[c] = np.bincount(key, minlength=NB)
        per_edges.append((ssc, ddc, w_arr, key))
    tiles = np.maximum(1, _ceil(kws.max(axis=0), 128)).astype(np.int64)
    tile_off = np.concatenate([[0], np.cumsum(tiles)])
    TOT_TILES = int(tile_off[-1])
    cfg.tiles, cfg.tile_off, cfg.TOT_TILES = tiles, tile_off, TOT_TILES
    cfg.TBMAX = int(tiles.max())

    per_core = []
    for c in range(P):
        ssc, ddc, w_arr, key = per_edges[c]
        order2 = np.argsort(key, kind="stable")
        sk = key[order2]
        grp_start = np.searchsorted(sk, np.arange(NB))
        rank = np.arange(sk.shape[0]) - grp_start[sk]
        slotpos = tile_off[sk] * 128 + rank
        TOT_SLOT = TOT_TILES * 128
        srel = np.zeros(TOT_SLOT, np.int16)
        drel = np.full(TOT_SLOT, 255, np.uint8)
        srel[slotpos] = rel_of[ssc[order2]]
        drel[slotpos] = (ddc[order2] - n0[c] - 128 * w_arr[order2]).astype(np.uint8)
        # wrapped gather indices: idx i of a (tile-aligned) run at [i%16, i//16]
        wr = np.ascontiguousarray(srel.reshape(TOT_SLOT // 16, 16).T)
        idx_wr = np.tile(wr, (8, 1))                                   # [128, TOT_SLOT//16]
        drel_pt = np.ascontiguousarray(drel.reshape(TOT_TILES, 128).T)  # [128, TOT_TILES]
        drelT = drel.reshape(1, TOT_TILES, 128).copy()                  # [1, TOT_TILES, 128]
        bd = np.full(NPAD2, 999.0, np.float32)
        rg = int(ranges[c])
        bd[:rg] = np.asarray(batch, np.int64)[n0[c]:n0[c + 1]].astype(np.float32)
        batch_wd = np.ascontiguousarray(bd.reshape(NW, 128).T)          # [128, NW]
        per_core.append(dict(idx_wr=idx_wr, drel_pt=drel_pt, drelT=drelT,
                             batch_wd=batch_wd, rg=rg))
    return per_core


def build_inputs(cfg, x, weights, per_core):
    (gru_w_ih, gru_w_hh, gru_b_ih, gru_b_hh, W1, att_src1, att_dst1, b1,
     W2, att_src2, att_dst2, b2, fc_w, fc_b) = weights
    P, NPAD2 = cfg.P, cfg.NPAD2
    bf = ml_dtypes.bfloat16

    # BC1 [64, 8]: cols 0:4 src-att coeffs per head, 4:8 dst-att
    BC1 = np.zeros((64, 8), np.float32)
    for h in range(4):
        Wh = W1[:, 64 * h:64 * (h + 1)]
        BC1[:, h] = Wh @ att_src1[h]
        BC1[:, 4 + h] = Wh @ att_dst1[h]
    # W1 block-diagonal pairs for transposed apply
    W12A = np.zeros((128, 128), np.float32)
    W12B = np.zeros((128, 128), np.float32)
    W12A[0:64, 0:64] = W1[:, 0:64]
    W12A[64:128, 64:128] = W1[:, 64:128]
    W12B[0:64, 0:64] = W1[:, 128:192]
    W12B[64:128, 64:128] = W1[:, 192:256]
    # W2 halves side by side; BC2 [128, 4]: cols 2h = [src|dst] coeffs, half h
    W2s = np.zeros((128, 128), np.float32)
    W2s[:, 0:64] = W2[0:128, :]
    W2s[:, 64:128] = W2[128:256, :]
    a2 = W2 @ att_src2[0]   # [256]
    d2 = W2 @ att_dst2[0]
    BC2 = np.zeros((128, 4), np.float32)
    BC2[:, 0] = a2[0:128]
    BC2[:, 1] = d2[0:128]
    BC2[:, 2] = a2[128:256]
    BC2[:, 3] = d2[128:256]

    com = dict(
        wihT=np.ascontiguousarray(gru_w_ih.T).astype(bf),               # [128,192]
        whrz=np.concatenate([gru_w_hh.T[:, 0:128],
                             (gru_b_ih + gru_b_hh)[None, 0:128]], 0).astype(bf),  # [65,128]
        whn=np.concatenate([gru_w_hh.T[:, 128:192],
                            gru_b_hh[None, 128:192]], 0).astype(bf),    # [65,64]
        bihn=np.ascontiguousarray(gru_b_ih[128:192].reshape(64, 1)).astype(np.float32),
        BC1=BC1.astype(bf),
        W12A=W12A.astype(bf), W12B=W12B.astype(bf),
        b1b=np.broadcast_to(b1, (128, 256)).astype(bf).copy(),
        W2s=W2s.astype(bf), BC2=BC2.astype(bf),
        b2b=np.broadcast_to(b2, (128, 64)).astype(bf).copy(),
        fcw=fc_w.astype(np.float32),
        fcbb=np.broadcast_to(fc_b, (128, 10)).astype(np.float32).copy(),
    )
    in_maps = []
    for c in range(P):
        pc = per_core[c]
        rg = pc["rg"]
        xp = np.zeros((NPAD2, cfg.T, cfg.D), np.float32)
        xp[:rg] = x[cfg.n0[c]:cfg.n0[c + 1]]
        xpT = np.ascontiguousarray(xp.transpose(1, 2, 0)).astype(bf)     # [8,128,NPAD2]
        m = dict(com)
        m.update(xpT=xpT, idx_wr=pc["idx_wr"], drel_pt=pc["drel_pt"],
                 drelT=pc["drelT"], batch_wd=pc["batch_wd"])
        in_maps.append(m)
    return in_maps


def build_kernel(cfg, dbg=False):
    P, T, NPAD2, NW, SH = cfg.P, cfg.T, cfg.NPAD2, cfg.NW, cfg.SH
    NT, NSHARD = cfg.NT, NSH
    tiles, tile_off, TOT_TILES = cfg.tiles, cfg.tile_off, cfg.TOT_TILES
    TOT_SLOT = TOT_TILES * 128
    rg_all = [list(range(P))]

    nc = bacc.Bacc("TRN2", target_bir_lowering=False, debug=False,
                   dynamic_dma_scratch_size=32768)
    # inputs
    xpT = nc.dram_tensor("xpT", [T, 128, NPAD2], BF16, kind="ExternalInput")
    wihT = nc.dram_tensor("wihT", [128, 192], BF16, kind="ExternalInput")
    whrz = nc.dram_tensor("whrz", [65, 128], BF16, kind="ExternalInput")
    whn = nc.dram_tensor("whn", [65, 64], BF16, kind="ExternalInput")
    bihn = nc.dram_tensor("bihn", [64, 1], F32, kind="ExternalInput")
    BC1 = nc.dram_tensor("BC1", [64, 8], BF16, kind="ExternalInput")
    W12A = nc.dram_tensor("W12A", [128, 128], BF16, kind="ExternalInput")
    W12B = nc.dram_tensor("W12B", [128, 128], BF16, kind="ExternalInput")
    b1b = nc.dram_tensor("b1b", [128, 256], BF16, kind="ExternalInput")
    W2s = nc.dram_tensor("W2s", [128, 128], BF16, kind="ExternalInput")
    BC2 = nc.dram_tensor("BC2", [128, 4], BF16, kind="ExternalInput")
    b2b = nc.dram_tensor("b2b", [128, 64], BF16, kind="ExternalInput")
    fcw = nc.dram_tensor("fcw", [64, 10], F32, kind="ExternalInput")
    fcbb = nc.dram_tensor("fcbb", [128, 10], F32, kind="ExternalInput")
    idx_wr = nc.dram_tensor("idx_wr", [128, TOT_SLOT // 16], I16, kind="ExternalInput")
    drel_pt = nc.dram_tensor("drel_pt", [128, TOT_TILES], U8, kind="ExternalInput")
    drelT = nc.dram_tensor("drelT", [1, TOT_TILES, 128], U8, kind="ExternalInput")
    batch_wd = nc.dram_tensor("batch_wd", [128, NW], F32, kind="ExternalInput")
    out = nc.dram_tensor("out", [128, 16], F32, kind="ExternalOutput")
    # internal dram
    xcat1l = nc.dram_tensor("xcat1l", [NPAD2, 128], BF16)
    table1 = nc.dram_tensor("table1", [P * NPAD2, 128], BF16, addr_space="Shared")
    tb1s = [nc.dram_tensor(f"tb1s{s}", [SH, 128], BF16) for s in range(NSHARD)]
    xcat2l = nc.dram_tensor("xcat2l", [NPAD2, 128], BF16)
    table2 = nc.dram_tensor("table2", [P * NPAD2, 128], BF16, addr_space="Shared")
    tb2s = [nc.dram_tensor(f"tb2s{s}", [SH, 128], BF16) for s in range(NSHARD)]
    arin = nc.dram_tensor("arin", [128, 65], F32)
    arout = nc.dram_tensor("arout", [128, 65], F32, addr_space="Shared")

    with tile.TileContext(nc) as tc:
        with tc.tile_pool(name="pers", bufs=1) as pers:
            # ---- persistent: weights, indices, iotas ----
            identb = pers.tile([128, 128], BF16)
            make_identity(nc, identb[:])
            iota_row = pers.tile([128, 128], U8)
            nc.gpsimd.iota(iota_row[:], pattern=[[1, 128]], base=0, channel_multiplier=0,
                           allow_small_or_imprecise_dtypes=True)
            iota_p = pers.tile([128, 1], F32)
            nc.gpsimd.iota(iota_p[:], pattern=[[0, 1]], base=0, channel_multiplier=1,
                           allow_small_or_imprecise_dtypes=True)
            wihT_s = pers.tile([128, 192], BF16)
            nc.sync.dma_start(out=wihT_s[:], in_=wihT[:])
            whrz_s = pers.tile([65, 128], BF16)
            nc.sync.dma_start(out=whrz_s[:], in_=whrz[:])
            whn_s = pers.tile([65, 64], BF16)
            nc.sync.dma_start(out=whn_s[:], in_=whn[:])
            bihn_s = pers.tile([64, 1], F32)
            nc.sync.dma_start(out=bihn_s[:], in_=bihn[:])
            BC1_s = pers.tile([64, 8], BF16)
            nc.sync.dma_start(out=BC1_s[:], in_=BC1[:])
            W12A_s = pers.tile([128, 128], BF16)
            nc.sync.dma_start(out=W12A_s[:], in_=W12A[:])
            W12B_s = pers.tile([128, 128], BF16)
            nc.sync.dma_start(out=W12B_s[:], in_=W12B[:])
            b1b_s = pers.tile([128, 256], BF16)
            nc.sync.dma_start(out=b1b_s[:], in_=b1b[:])
            W2s_s = pers.tile([128, 128], BF16)
            nc.sync.dma_start(out=W2s_s[:], in_=W2s[:])
            BC2_s = pers.tile([128, 4], BF16)
            nc.sync.dma_start(out=BC2_s[:], in_=BC2[:])
            b2b_s = pers.tile([128, 64], BF16)
            nc.sync.dma_start(out=b2b_s[:], in_=b2b[:])
            fcw_s = pers.tile([64, 10], F32)
            nc.sync.dma_start(out=fcw_s[:], in_=fcw[:])
            fcbb_s = pers.tile([128, 10], F32)
            nc.sync.dma_start(out=fcbb_s[:], in_=fcbb[:])
            idx_sb = pers.tile([128, TOT_SLOT // 16], I16)
            nc.sync.dma_start(out=idx_sb[:], in_=idx_wr[:])
            drel_sb = pers.tile([128, TOT_TILES], U8)
            nc.sync.dma_start(out=drel_sb[:], in_=drel_pt[:])
            batch_sb = pers.tile([128, NW], F32)
            nc.sync.dma_start(out=batch_sb[:], in_=batch_wd[:])
            adt1_sb = pers.tile([128, NW, 4], BF16)
            adt2_sb = pers.tile([128, NW, 1], BF16)
            nidx_regs = {}
            for b in range(NW * NSHARD):
                Tb = int(tiles[b])
                for q0 in range(0, Tb, 8):
                    n = min(8, Tb - q0) * 128
                    if n not in nidx_regs:
                        nidx_regs[n] = nc.gpsimd.to_reg(n)

            # ---- phase 1: GRU -> xcat1l (h|asrc), adt1_sb ----
            with (
                tc.tile_pool(name="gx", bufs=2) as gx,
                tc.tile_pool(name="gh", bufs=2) as gh,
                tc.tile_pool(name="gv", bufs=3) as gv,
                tc.tile_pool(name="gp1", bufs=1, space="PSUM") as gp1,
                tc.tile_pool(name="gp2", bufs=1, space="PSUM") as gp2,
                tc.tile_pool(name="gp3", bufs=1, space="PSUM") as gp3,
                tc.tile_pool(name="gp4", bufs=2, space="PSUM") as gp4,
            ):
                for it in range(NT):
                    xt8 = gx.tile([128, T, GT], BF16, tag="xt8")
                    for t in range(T):
                        nc.sync.dma_start(out=xt8[:, t, :],
                                          in_=xpT[t, :, it * GT:(it + 1) * GT])
                    hT = gh.tile([65, GT], BF16, tag="hT")
                    nc.vector.memset(hT[0:64, :], 0.0)
                    nc.vector.memset(hT[64:65, :], 1.0)
                    for t in range(T):
                        prz = gp1.tile([64, 2 * GT], F32, tag="prz")
                        nc.tensor.matmul(out=prz[:, 0:GT], lhsT=wihT_s[:, 0:64],
                                         rhs=xt8[:, t, :], start=True, stop=False)
                        nc.tensor.matmul(out=prz[:, 0:GT], lhsT=whrz_s[:, 0:64],
                                         rhs=hT[:], start=False, stop=True)
                        nc.tensor.matmul(out=prz[:, GT:2 * GT], lhsT=wihT_s[:, 64:128],
                                         rhs=xt8[:, t, :], start=True, stop=False)
                        nc.tensor.matmul(out=prz[:, GT:2 * GT], lhsT=whrz_s[:, 64:128],
                                         rhs=hT[:], start=False, stop=True)
                        pin = gp2.tile([64, GT], F32, tag="pin")
                        nc.tensor.matmul(out=pin[:], lhsT=wihT_s[:, 128:192],
                                         rhs=xt8[:, t, :], start=True, stop=True)
                        phn = gp3.tile([64, GT], F32, tag="phn")
                        nc.tensor.matmul(out=phn[:], lhsT=whn_s[:], rhs=hT[:],
                                         start=True, stop=True)
                        rz = gv.tile([64, 2 * GT], BF16, tag="rz")
                        nc.scalar.activation(out=rz[:], in_=prz[:], func=AF.Sigmoid)
                        tmp = gv.tile([64, GT], BF16, tag="tmp")
                        nc.vector.tensor_mul(out=tmp[:], in0=rz[:, 0:GT], in1=phn[:])
                        t3 = gv.tile([64, GT], F32, tag="t3")
                        nc.vector.tensor_add(out=t3[:], in0=pin[:], in1=tmp[:])
                        nh = gv.tile([64, GT], BF16, tag="nh")
                        nc.scalar.activation(out=nh[:], in_=t3[:], func=AF.Tanh,
                                             bias=bihn_s[:, 0:1])
                        s1 = gv.tile([64, GT], BF16, tag="s1")
                        nc.vector.tensor_sub(out=s1[:], in0=hT[0:64, :], in1=nh[:])
                        s2 = gv.tile([64, GT], BF16, tag="s2")
                        nc.vector.tensor_mul(out=s2[:], in0=rz[:, GT:2 * GT], in1=s1[:])
                        nc.vector.tensor_add(out=hT[0:64, :], in0=nh[:], in1=s2[:])
                    for cc in range(GT // 128):
                        w = it * (GT // 128) + cc
                        pt = gp4.tile([128, 64], BF16, tag="pt")
                        nc.tensor.transpose(out=pt[:],
                                            in_=hT[0:64, cc * 128:(cc + 1) * 128],
                                            identity=identb[0:64, 0:64])
                        pat = gp4.tile([128, 8], F32, tag="pat")
                        nc.tensor.matmul(out=pat[:], lhsT=hT[0:64, cc * 128:(cc + 1) * 128],
                                         rhs=BC1_s[:], start=True, stop=True)
                        xc = gv.tile([128, 128], BF16, tag="xc")
                        nc.vector.memset(xc[:, 0:1], 1.0)
                        nc.scalar.copy(out=xc[:, 1:65], in_=pt[:])
                        nc.vector.tensor_copy(out=xc[:, 65:69], in_=pat[:, 0:4])
                        nc.vector.memset(xc[:, 69:128], 0.0)
                        nc.vector.tensor_copy(out=adt1_sb[:, w, :], in_=pat[:, 4:8])
                        nc.sync.dma_start(out=xcat1l[w * 128:(w + 1) * 128, :], in_=xc[:])

            # ---- AllGather table1, split into shards ----
            nc.gpsimd.collective_compute(
                "AllGather", OP.bypass, replica_groups=rg_all,
                ins=[xcat1l[:]], outs=[table1[:]])
            for s in range(NSHARD):
                for ci in range(2):
                    nc.sync.dma_start(
                        out=tb1s[s][ci * NPAD2:(ci + 1) * NPAD2, :],
                        in_=table1[(2 * s + ci) * NPAD2:(2 * s + ci + 1) * NPAD2, :])

            # ---- GAT layer over windows (shared for layer 1 / layer 2) ----
            def gat_windows(tbls, adt_sb, nheads, payw, post_fn):
                """payw: scatter matmul width (4+256 for L1, 1+64 for L2)."""
                with (
                    tc.tile_pool(name="pg", bufs=3) as pg,
                    tc.tile_pool(name="pS", bufs=2) as pS,
                    tc.tile_pool(name="pd", bufs=2) as pd,
                    tc.tile_pool(name="pu", bufs=2) as pu,
                    tc.tile_pool(name="pM", bufs=2) as pM,
                    tc.tile_pool(name="pw", bufs=2) as pw,
                    tc.tile_pool(name="ps2", bufs=2, space="PSUM") as ps2,
                    tc.tile_pool(name="psA", bufs=1, space="PSUM") as psA,
                    tc.tile_pool(name="psB", bufs=1, space="PSUM") as psB,
                ):
                    for w in range(NW):
                        pblk = ps2.tile([128, payw], F32, tag="pblk")
                        first = True
                        for s in range(NSHARD):
                            b = w * NSHARD + s
                            Tb = int(tiles[b])
                            t0 = int(tile_off[b])
                            g = pg.tile([128, Tb, 128], BF16, tag=f"g{Tb}")
                            for q0 in range(0, Tb, 8):
                                qn = min(8, Tb - q0)
                                nc.gpsimd.dma_gather(
                                    out_ap=g[:, q0:q0 + qn, :], in_ap=tbls[s][:],
                                    idxs_ap=idx_sb[:, (t0 + q0) * 8:(t0 + q0 + qn) * 8],
                                    num_idxs=qn * 128, num_idxs_reg=nidx_regs[qn * 128],
                                    elem_size=128)
                            S = pS.tile([128, Tb, 128], BF16, tag=f"S{Tb}")
                            nc.vector.tensor_tensor(
                                out=S[:],
                                in0=iota_row[:].unsqueeze(1).broadcast_to([128, Tb, 128]),
                                in1=drel_sb[:, t0:t0 + Tb].unsqueeze(2).broadcast_to([128, Tb, 128]),
                                op=OP.is_equal)
                            drT = pd.tile([128, Tb, 128], U8, tag=f"dT{Tb}")
                            nc.sync.dma_start(
                                out=drT[:],
                                in_=drelT[0:1, t0:t0 + Tb, :].partition_broadcast(128))
                            Sd = pS.tile([128, Tb, 128], BF16, tag=f"Sd{Tb}")
                            nc.vector.tensor_scalar(
                                out=Sd[:], in0=drT[:], scalar1=iota_p[:, 0:1],
                                scalar2=None, op0=OP.is_equal)
                            padp = psA.tile([128, Tb * nheads], F32, tag="padp")
                            for t in range(Tb):
                                nc.tensor.matmul(
                                    out=padp[:, t * nheads:(t + 1) * nheads],
                                    lhsT=Sd[:, t, :], rhs=adt_sb[:, w, :],
                                    start=True, stop=True)
                            u = pu.tile([128, Tb, nheads], F32, tag=f"u{Tb}")
                            nc.vector.tensor_add(
                                out=u[:], in0=g[:, :, 65:65 + nheads],
                                in1=padp[:].rearrange("p (t c) -> p t c", t=Tb))
                            e1 = pu.tile([128, Tb, nheads], BF16, tag=f"e1{Tb}")
                            nc.scalar.activation(out=e1[:], in_=u[:], func=AF.Exp)
                            e2 = pu.tile([128, Tb, nheads], BF16, tag=f"e2{Tb}")
                            nc.scalar.activation(out=e2[:], in_=u[:], func=AF.Exp, scale=0.2)
                            ee = pu.tile([128, Tb, nheads], BF16, tag=f"ee{Tb}")
                            nc.vector.tensor_tensor(out=ee[:], in0=e1[:], in1=e2[:], op=OP.max)
                            M = pM.tile([128, Tb, payw], BF16, tag=f"M{Tb}")
                            if nheads == 4:
                                nc.vector.tensor_tensor(
                                    out=M[:].rearrange("p t (h c) -> p t h c", h=4),
                                    in0=g[:, :, 0:65].unsqueeze(2).broadcast_to([128, Tb, 4, 65]),
                                    in1=ee[:].unsqueeze(3).broadcast_to([128, Tb, 4, 65]),
                                    op=OP.mult)
                            else:
                                nc.vector.tensor_tensor(
                                    out=M[:],
                                    in0=g[:, :, 0:65],
                                    in1=ee[:].to_broadcast([128, Tb, 65]),
                                    op=OP.mult)
                            for t in range(Tb):
                                nc.tensor.matmul(
                                    out=pblk[:], lhsT=S[:, t, :], rhs=M[:, t, :],
                                    start=first, stop=(s == NSHARD - 1 and t == Tb - 1))
                                first = False
                        post_fn(w, pblk, pw, psB)

            # ---- layer 1 post: h1 = relu(z/denom @ W1 + b1) -> xcat2, adt2 ----
            def post1(w, pblk, pw, psB):
                dn = pw.tile([128, 4], F32, tag="dn")
                nc.vector.tensor_scalar(
                    out=dn[:], in0=pblk[:].rearrange("p (h c) -> p h c", h=4)[:, :, 0],
                    scalar1=1e-12, scalar2=None, op0=OP.max)
                rec = pw.tile([128, 4], F32, tag="rec")
                nc.vector.reciprocal(out=rec[:], in_=dn[:])
                zn = pw.tile([128, 256], BF16, tag="zn")
                nc.vector.tensor_tensor(
                    out=zn[:].rearrange("p (h c) -> p h c", h=4),
                    in0=pblk[:].rearrange("p (h c) -> p h c", h=4)[:, :, 1:65],
                    in1=rec[:].unsqueeze(2).broadcast_to([128, 4, 64]),
                    op=OP.mult)
                tpz = pw.tile([128, 256], BF16, tag="tpz")
                for half in range(2):
                    ptp = psB.tile([128, 128], BF16, tag="ptp")
                    nc.tensor.transpose(out=ptp[:], in_=zn[:, 128 * half:128 * (half + 1)],
                                        identity=identb[:])
                    nc.scalar.copy(out=tpz[:, 128 * half:128 * (half + 1)], in_=ptp[:])
                h1p = psB.tile([128, 256], F32, tag="h1p")
                nc.tensor.matmul(out=h1p[:, 0:128], lhsT=tpz[:, 0:128], rhs=W12A_s[:],
                                 start=True, stop=True)
                nc.tensor.matmul(out=h1p[:, 128:256], lhsT=tpz[:, 128:256], rhs=W12B_s[:],
                                 start=True, stop=True)
                h1 = pw.tile([128, 256], BF16, tag="h1")
                nc.vector.tensor_add(out=h1[:], in0=h1p[:], in1=b1b_s[:])
                nc.vector.tensor_scalar(out=h1[:], in0=h1[:], scalar1=0.0,
                                        scalar2=None, op0=OP.max)
                th1 = pw.tile([128, 256], BF16, tag="th1")
                for half in range(2):
                    ptp = psB.tile([128, 128], BF16, tag="ptp")
                    nc.tensor.transpose(out=ptp[:], in_=h1[:, 128 * half:128 * (half + 1)],
                                        identity=identb[:])
                    nc.scalar.copy(out=th1[:, 128 * half:128 * (half + 1)], in_=ptp[:])
                xsc = psB.tile([128, 66], F32, tag="xsc")
                xs2p = xsc[:, 0:64]
                pat2 = xsc[:, 64:66]
                for half in range(2):
                    nc.tensor.matmul(out=xs2p, lhsT=th1[:, 128 * half:128 * (half + 1)],
                                     rhs=W2s_s[:, 64 * half:64 * (half + 1)],
                                     start=(half == 0), stop=(half == 1))
                    nc.tensor.matmul(out=pat2, lhsT=th1[:, 128 * half:128 * (half + 1)],
                                     rhs=BC2_s[:, 2 * half:2 * half + 2],
                                     start=(half == 0), stop=(half == 1))
                xc2 = pw.tile([128, 128], BF16, tag="xc2")
                nc.vector.memset(xc2[:, 0:1], 1.0)
                nc.scalar.copy(out=xc2[:, 1:65], in_=xs2p)
                nc.vector.tensor_copy(out=xc2[:, 65:66], in_=pat2[:, 0:1])
                nc.vector.memset(xc2[:, 66:128], 0.0)
                nc.vector.tensor_copy(out=adt2_sb[:, w, :], in_=pat2[:, 1:2])
                nc.sync.dma_start(out=xcat2l[w * 128:(w + 1) * 128, :], in_=xc2[:])

            gat_windows(tb1s, adt1_sb, 4, 260, post1)

            nc.gpsimd.collective_compute(
                "AllGather", OP.bypass, replica_groups=rg_all,
                ins=[xcat2l[:]], outs=[table2[:]])
            for s in range(NSHARD):
                for ci in range(2):
                    nc.sync.dma_start(
                        out=tb2s[s][ci * NPAD2:(ci + 1) * NPAD2, :],
                        in_=table2[(2 * s + ci) * NPAD2:(2 * s + ci + 1) * NPAD2, :])

            # ---- layer 2 post: pooling into ppool psum ----
            pp_ctx = tc.tile_pool(name="psPool", bufs=1, space="PSUM")
            psPool = pp_ctx.__enter__()
            ppool = psPool.tile([128, 65], F32)

            def post2(w, pblk, pw, psB):
                dn2 = pw.tile([128, 1], F32, tag="dn2")
                nc.vector.tensor_scalar(out=dn2[:], in0=pblk[:, 0:1], scalar1=1e-12,
                                        scalar2=None, op0=OP.max)
                rec2 = pw.tile([128, 1], F32, tag="rec2")
                nc.vector.reciprocal(out=rec2[:], in_=dn2[:])
                ph = pw.tile([128, 65], BF16, tag="ph")
                nc.vector.tensor_scalar(out=ph[:, 0:64], in0=pblk[:, 1:65],
                                        scalar1=rec2[:, 0:1], scalar2=None, op0=OP.mult)
                nc.vector.tensor_add(out=ph[:, 0:64], in0=ph[:, 0:64], in1=b2b_s[:])
                nc.vector.tensor_scalar(out=ph[:, 0:64], in0=ph[:, 0:64], scalar1=0.0,
                                        scalar2=None, op0=OP.max)
                nc.vector.memset(ph[:, 64:65], 1.0)
                Sb = pw.tile([128, 128], BF16, tag="Sb")
                nc.vector.tensor_scalar(out=Sb[:], in0=iota_row[:],
                                        scalar1=batch_sb[:, w:w + 1], scalar2=None,
                                        op0=OP.is_equal)
                nc.tensor.matmul(out=ppool[:], lhsT=Sb[:], rhs=ph[:],
                                 start=(w == 0), stop=(w == NW - 1))

            gat_windows(tb2s, adt2_sb, 1, 65, post2)

            # ---- tail: AllReduce pools, fc, log_softmax ----
            with tc.tile_pool(name="rpre", bufs=1) as rpre:
                pr = rpre.tile([128, 65], F32)
                nc.scalar.copy(out=pr[:], in_=ppool[:])
                nc.sync.dma_start(out=arin[:], in_=pr[:])
            pp_ctx.__exit__(None, None, None)
            with (
                tc.tile_pool(name="r5", bufs=1) as r5,
                tc.tile_pool(name="r5q", bufs=1, space="PSUM") as r5q,
            ):
                tc.strict_bb_all_engine_barrier()
                nc.gpsimd.collective_compute(
                    "AllReduce", OP.add, replica_groups=rg_all,
                    ins=[arin[:]], outs=[arout[:]])
                tc.strict_bb_all_engine_barrier()
                ar = r5.tile([128, 65], F32)
                nc.sync.dma_start(out=ar[:], in_=arout[:])
                cm = r5.tile([128, 1], F32)
                nc.vector.tensor_scalar(out=cm[:], in0=ar[:, 64:65], scalar1=1.0,
                                        scalar2=None, op0=OP.max)
                cr = r5.tile([128, 1], F32)
                nc.vector.reciprocal(out=cr[:], in_=cm[:])
                gf = r5.tile([128, 64], F32)
                nc.vector.tensor_scalar(out=gf[:], in0=ar[:, 0:64], scalar1=cr[:, 0:1],
                                        scalar2=None, op0=OP.mult)
                identf = r5.tile([128, 128], F32)
                make_identity(nc, identf[:])
                pgt = r5q.tile([64, 128], F32)
                nc.tensor.transpose(out=pgt[:], in_=gf[:], identity=identf[:])
                gfT = r5.tile([64, 128], F32)
                nc.scalar.copy(out=gfT[:], in_=pgt[:])
                plg = r5q.tile([128, 10], F32)
                nc.tensor.matmul(out=plg[:], lhsT=gfT[:], rhs=fcw_s[:], start=True, stop=True)
                lg = r5.tile([128, 16], F32)
                nc.vector.tensor_add(out=lg[:, 0:10], in0=plg[:], in1=fcbb_s[:])
                mx = r5.tile([128, 1], F32)
                nc.vector.reduce_max(out=mx[:], in_=lg[:, 0:10], axis=mybir.AxisListType.X)
                tsh = r5.tile([128, 16], F32)
                nc.vector.tensor_scalar(out=tsh[:, 0:10], in0=lg[:, 0:10],
                                        scalar1=mx[:, 0:1], scalar2=None, op0=OP.subtract)
                exs = r5.tile([128, 16], F32)
                se = r5.tile([128, 1], F32)
                nc.scalar.activation(out=exs[:, 0:10], in_=tsh[:, 0:10], func=AF.Exp,
                                     accum_out=se[:])
                ln = r5.tile([128, 1], F32)
                nc.scalar.activation(out=ln[:], in_=se[:], func=AF.Ln)
                res = r5.tile([128, 16], F32)
                nc.vector.memset(res[:], 0.0)
                nc.vector.tensor_scalar(out=res[:, 0:10], in0=tsh[:, 0:10],
                                        scalar1=ln[:, 0:1], scalar2=None, op0=OP.subtract)
                nc.sync.dma_start(out=out[:], in_=res[:])
    nc.compile()
    return nc


# ---------------- self-contained entry point ----------------
_CACHE = {}


def kernel(**inputs):
    """Full DAGNN forward. Takes the unsharded inputs from setup_inputs();
    returns log-softmax output [num_graphs, 10] float32."""
    x = np.asarray(inputs["x"], np.float32)
    edge_index = np.asarray(inputs["edge_index"])
    batch = np.asarray(inputs["batch"])
    G = int(inputs["num_graphs"])
    weights = [np.asarray(inputs[k], np.float32) for k in (
        "gru_w_ih", "gru_w_hh", "gru_b_ih", "gru_b_hh",
        "W1", "att_src1", "att_dst1", "b1",
        "W2", "att_src2", "att_dst2", "b2", "fc_w", "fc_b")]
    N = x.shape[0]
    E = edge_index.shape[1]
    P = 8

    from concourse.bass_utils import run_bass_kernel_spmd
    cfg = Cfg(N, E, G, P)
    per_core = host_prep(cfg, edge_index, batch)
    in_maps = build_inputs(cfg, x, weights, per_core)
    key = (N, E, G, P, cfg.NPAD2, cfg.TOT_TILES, tuple(cfg.tiles[:8]))
    if key not in _CACHE:
        _CACHE[key] = build_kernel(cfg)
    nc = _CACHE[key]
    res = run_bass_kernel_spmd(nc, in_maps, core_ids=list(range(P)))
    out = np.asarray(res.results[0]["out"][:G, :10], np.float32)
    return out


# revision 21
# speedup vs baseline: 1.1515x; 1.0986x over previous
"""DAGNN (GRU + 2xGAT + mean-pool + fc + log_softmax) on 8 TRN2 cores via Bass/Tile.

Sharding: nodes split across cores by dst-range (edges sorted by dst, split at
dst boundaries), so each core's GRU computes exactly the h/attention values its
GAT dst windows need locally. Edge payload gathers use batched dma_gather from
a 4-way row-sharded bf16 table (int16 index limit); per-window dst attention
terms are expanded on-chip via selection-matrix matmuls. Feature tables are
AllGathered; graph pooling partial sums are AllReduced.
"""
import sys
import numpy as np

sys.path.insert(0, "/opt/trn_rl_repo")

import ml_dtypes
import concourse.bass as bass
import concourse.bacc as bacc
import concourse.mybir as mybir
import concourse.tile as tile
from concourse.masks import make_identity

F32 = mybir.dt.float32
BF16 = mybir.dt.bfloat16
I16 = mybir.dt.int16
U8 = mybir.dt.uint8
AF = mybir.ActivationFunctionType
OP = mybir.AluOpType

NSH = 4          # table row shards (int16 gather index limit)
GT = 512         # GRU node tile


def _ceil(a, b):
    return -(-a // b)


class Cfg:
    def __init__(self, N, E, G, P):
        self.N, self.E, self.G, self.P = N, E, G, P
        self.T, self.D, self.H = 8, 128, 64
        self.HEADS, self.C1, self.C2 = 4, 256, 64


def host_prep(cfg, edge_index, batch):
    N, E, P = cfg.N, cfg.E, cfg.P
    src = np.concatenate([np.asarray(edge_index[0], np.int64), np.arange(N, dtype=np.int64)])
    dst = np.concatenate([np.asarray(edge_index[1], np.int64), np.arange(N, dtype=np.int64)])
    order = np.argsort(dst, kind="stable")
    ss, dd = src[order], dst[order]
    Etot = ss.shape[0]

    bounds = [0]
    for k in range(1, P):
        pos = (k * Etot) // P
        while pos < Etot and dd[pos] == dd[pos - 1]:
            pos += 1
        bounds.append(pos)
    bounds.append(Etot)
    n0 = np.zeros(P + 1, np.int64)
    n0[P] = N
    for c in range(1, P):
        n0[c] = dd[bounds[c]]
    ranges = np.diff(n0)
    NPAD2 = _ceil(int(ranges.max()), GT) * GT
    NW = NPAD2 // 128
    SH = (P * NPAD2) // NSH
    assert SH - 1 <= 32767, f"shard too large for int16: {SH}"
    cfg.n0, cfg.NPAD2, cfg.NW, cfg.SH = n0, NPAD2, NW, SH
    cfg.NT = NPAD2 // GT

    owner = np.searchsorted(n0[1:P], np.arange(N), side="right")
    g2r = owner * NPAD2 + (np.arange(N) - n0[owner])
    shard_of = (g2r // SH).astype(np.int64)
    rel_of = (g2r % SH).astype(np.int16)

    # pass 1: per-(window, shard) edge counts per core -> uniform tile counts
    NB = NW * NSH
    kws = np.zeros((P, NB), np.int64)
    per_edges = []
    for c in range(P):
        sl = slice(bounds[c], bounds[c + 1])
        ssc, ddc = ss[sl], dd[sl]
        w_arr = (ddc - n0[c]) // 128
        s_arr = shard_of[ssc]
        key = (w_arr * NSH + s_arr).astype(np.int64)
        kws[c] = np.bincount(key, minlength=NB)
        per_edges.append((ssc, ddc, w_arr, key))
    tiles = np.maximum(1, _ceil(kws.max(axis=0), 128)).astype(np.int64)
    tile_off = np.concatenate([[0], np.cumsum(tiles)])
    TOT_TILES = int(tile_off[-1])
    cfg.tiles, cfg.tile_off, cfg.TOT_TILES = tiles, tile_off, TOT_TILES
    cfg.TBMAX = int(tiles.max())

    per_core = []
    for c in range(P):
        ssc, ddc, w_arr, key = per_edges[c]
        order2 = np.argsort(key, kind="stable")
        sk = key[order2]
        grp_start = np.searchsorted(sk, np.arange(NB))
        rank = np.arange(sk.shape[0]) - grp_start[sk]
        slotpos = tile_off[sk] * 128 + rank
        TOT_SLOT = TOT_TILES * 128
        srel = np.zeros(TOT_SLOT, np.int16)
        drel = np.full(TOT_SLOT, 255, np.uint8)
        srel[slotpos] = rel_of[ssc[order2]]
        drel[slotpos] = (ddc[order2] - n0[c] - 128 * w_arr[order2]).astype(np.uint8)
        # wrapped gather indices: idx i of a (tile-aligned) run at [i%16, i//16]
        wr = np.ascontiguousarray(srel.reshape(TOT_SLOT // 16, 16).T)
        idx_wr = np.tile(wr, (8, 1))                                   # [128, TOT_SLOT//16]
        drel_pt = np.ascontiguousarray(drel.reshape(TOT_TILES, 128).T)  # [128, TOT_TILES]
        drelT = drel.reshape(1, TOT_TILES, 128).copy()                  # [1, TOT_TILES, 128]
        bd = np.full(NPAD2, 999.0, np.float32)
        rg = int(ranges[c])
        bd[:rg] = np.asarray(batch, np.int64)[n0[c]:n0[c + 1]].astype(np.float32)
        batch_wd = np.ascontiguousarray(bd.reshape(NW, 128).T)          # [128, NW]
        per_core.append(dict(idx_wr=idx_wr, drel_pt=drel_pt, drelT=drelT,
                             batch_wd=batch_wd, rg=rg))
    return per_core


def build_inputs(cfg, x, weights, per_core):
    (gru_w_ih, gru_w_hh, gru_b_ih, gru_b_hh, W1, att_src1, att_dst1, b1,
     W2, att_src2, att_dst2, b2, fc_w, fc_b) = weights
    P, NPAD2 = cfg.P, cfg.NPAD2
    bf = ml_dtypes.bfloat16

    # BC1 [64, 8]: cols 0:4 src-att coeffs per head, 4:8 dst-att
    BC1 = np.zeros((64, 8), np.float32)
    for h in range(4):
        Wh = W1[:, 64 * h:64 * (h + 1)]
        BC1[:, h] = Wh @ att_src1[h]
        BC1[:, 4 + h] = Wh @ att_dst1[h]
    # W1 block-diagonal pairs for transposed apply
    W12A = np.zeros((128, 128), np.float32)
    W12B = np.zeros((128, 128), np.float32)
    W12A[0:64, 0:64] = W1[:, 0:64]
    W12A[64:128, 64:128] = W1[:, 64:128]
    W12B[0:64, 0:64] = W1[:, 128:192]
    W12B[64:128, 64:128] = W1[:, 192:256]
    # W2 halves side by side; BC2 [128, 4]: cols 2h = [src|dst] coeffs, half h
    W2s = np.zeros((128, 128), np.float32)
    W2s[:, 0:64] = W2[0:128, :]
    W2s[:, 64:128] = W2[128:256, :]
    a2 = W2 @ att_src2[0]   # [256]
    d2 = W2 @ att_dst2[0]
    BC2 = np.zeros((128, 4), np.float32)
    BC2[:, 0] = a2[0:128]
    BC2[:, 1] = d2[0:128]
    BC2[:, 2] = a2[128:256]
    BC2[:, 3] = d2[128:256]

    com = dict(
        wihT=np.ascontiguousarray(gru_w_ih.T).astype(bf),               # [128,192]
        whrz=np.concatenate([gru_w_hh.T[:, 0:128],
                             (gru_b_ih + gru_b_hh)[None, 0:128]], 0).astype(bf),  # [65,128]
        whn=np.concatenate([gru_w_hh.T[:, 128:192],
                            gru_b_hh[None, 128:192]], 0).astype(bf),    # [65,64]
        bihn=np.ascontiguousarray(gru_b_ih[128:192].reshape(64, 1)).astype(np.float32),
        BC1=BC1.astype(bf),
        W12A=W12A.astype(bf), W12B=W12B.astype(bf),
        b1b=np.broadcast_to(b1, (128, 256)).astype(bf).copy(),
        W2s=W2s.astype(bf), BC2=BC2.astype(bf),
        b2b=np.broadcast_to(b2, (128, 64)).astype(bf).copy(),
        fcw=fc_w.astype(np.float32),
        fcbb=np.broadcast_to(fc_b, (128, 10)).astype(np.float32).copy(),
    )
    in_maps = []
    for c in range(P):
        pc = per_core[c]
        rg = pc["rg"]
        xp = np.zeros((NPAD2, cfg.T, cfg.D), np.float32)
        xp[:rg] = x[cfg.n0[c]:cfg.n0[c + 1]]
        xpT = np.ascontiguousarray(xp.transpose(1, 2, 0)).astype(bf)     # [8,128,NPAD2]
        m = dict(com)
        m.update(xpT=xpT, idx_wr=pc["idx_wr"], drel_pt=pc["drel_pt"],
                 drelT=pc["drelT"], batch_wd=pc["batch_wd"])
        in_maps.append(m)
    return in_maps


def build_kernel(cfg, dbg=False):
    P, T, NPAD2, NW, SH = cfg.P, cfg.T, cfg.NPAD2, cfg.NW, cfg.SH
    NT, NSHARD = cfg.NT, NSH
    tiles, tile_off, TOT_TILES = cfg.tiles, cfg.tile_off, cfg.TOT_TILES
    TOT_SLOT = TOT_TILES * 128
    rg_all = [list(range(P))]

    nc = bacc.Bacc("TRN2", target_bir_lowering=False, debug=False,
                   dynamic_dma_scratch_size=32768)
    # inputs
    xpT = nc.dram_tensor("xpT", [T, 128, NPAD2], BF16, kind="ExternalInput")
    wihT = nc.dram_tensor("wihT", [128, 192], BF16, kind="ExternalInput")
    whrz = nc.dram_tensor("whrz", [65, 128], BF16, kind="ExternalInput")
    whn = nc.dram_tensor("whn", [65, 64], BF16, kind="ExternalInput")
    bihn = nc.dram_tensor("bihn", [64, 1], F32, kind="ExternalInput")
    BC1 = nc.dram_tensor("BC1", [64, 8], BF16, kind="ExternalInput")
    W12A = nc.dram_tensor("W12A", [128, 128], BF16, kind="ExternalInput")
    W12B = nc.dram_tensor("W12B", [128, 128], BF16, kind="ExternalInput")
    b1b = nc.dram_tensor("b1b", [128, 256], BF16, kind="ExternalInput")
    W2s = nc.dram_tensor("W2s", [128, 128], BF16, kind="ExternalInput")
    BC2 = nc.dram_tensor("BC2", [128, 4], BF16, kind="ExternalInput")
    b2b = nc.dram_tensor("b2b", [128, 64], BF16, kind="ExternalInput")
    fcw = nc.dram_tensor("fcw", [64, 10], F32, kind="ExternalInput")
    fcbb = nc.dram_tensor("fcbb", [128, 10], F32, kind="ExternalInput")
    idx_wr = nc.dram_tensor("idx_wr", [128, TOT_SLOT // 16], I16, kind="ExternalInput")
    drel_pt = nc.dram_tensor("drel_pt", [128, TOT_TILES], U8, kind="ExternalInput")
    drelT = nc.dram_tensor("drelT", [1, TOT_TILES, 128], U8, kind="ExternalInput")
    batch_wd = nc.dram_tensor("batch_wd", [128, NW], F32, kind="ExternalInput")
    out = nc.dram_tensor("out", [128, 16], F32, kind="ExternalOutput")
    # internal dram
    xcat1l = nc.dram_tensor("xcat1l", [NPAD2, 128], BF16)
    table1 = nc.dram_tensor("table1", [P * NPAD2, 128], BF16, addr_space="Shared")
    tb1s = [nc.dram_tensor(f"tb1s{s}", [SH, 128], BF16) for s in range(NSHARD)]
    xcat2l = nc.dram_tensor("xcat2l", [NPAD2, 128], BF16)
    table2 = nc.dram_tensor("table2", [P * NPAD2, 128], BF16, addr_space="Shared")
    tb2s = [nc.dram_tensor(f"tb2s{s}", [SH, 128], BF16) for s in range(NSHARD)]
    arin = nc.dram_tensor("arin", [128, 65], F32)
    arout = nc.dram_tensor("arout", [128, 65], F32, addr_space="Shared")

    with tile.TileContext(nc) as tc:
        with tc.tile_pool(name="pers", bufs=1) as pers:
            # ---- persistent: weights, indices, iotas ----
            identb = pers.tile([128, 128], BF16)
            make_identity(nc, identb[:])
            iota_row = pers.tile([128, 128], U8)
            nc.gpsimd.iota(iota_row[:], pattern=[[1, 128]], base=0, channel_multiplier=0,
                           allow_small_or_imprecise_dtypes=True)
            iota_p = pers.tile([128, 1], F32)
            nc.gpsimd.iota(iota_p[:], pattern=[[0, 1]], base=0, channel_multiplier=1,
                           allow_small_or_imprecise_dtypes=True)
            wihT_s = pers.tile([128, 192], BF16)
            nc.sync.dma_start(out=wihT_s[:], in_=wihT[:])
            whrz_s = pers.tile([65, 128], BF16)
            nc.sync.dma_start(out=whrz_s[:], in_=whrz[:])
            whn_s = pers.tile([65, 64], BF16)
            nc.sync.dma_start(out=whn_s[:], in_=whn[:])
            bihn_s = pers.tile([64, 1], F32)
            nc.sync.dma_start(out=bihn_s[:], in_=bihn[:])
            BC1_s = pers.tile([64, 8], BF16)
            nc.sync.dma_start(out=BC1_s[:], in_=BC1[:])
            W12A_s = pers.tile([128, 128], BF16)
            nc.sync.dma_start(out=W12A_s[:], in_=W12A[:])
            W12B_s = pers.tile([128, 128], BF16)
            nc.sync.dma_start(out=W12B_s[:], in_=W12B[:])
            b1b_s = pers.tile([128, 256], BF16)
            nc.sync.dma_start(out=b1b_s[:], in_=b1b[:])
            W2s_s = pers.tile([128, 128], BF16)
            nc.sync.dma_start(out=W2s_s[:], in_=W2s[:])
            BC2_s = pers.tile([128, 4], BF16)
            nc.sync.dma_start(out=BC2_s[:], in_=BC2[:])
            b2b_s = pers.tile([128, 64], BF16)
            nc.sync.dma_start(out=b2b_s[:], in_=b2b[:])
            fcw_s = pers.tile([64, 10], F32)
            nc.sync.dma_start(out=fcw_s[:], in_=fcw[:])
            fcbb_s = pers.tile([128, 10], F32)
            nc.sync.dma_start(out=fcbb_s[:], in_=fcbb[:])
            idx_sb = pers.tile([128, TOT_SLOT // 16], I16)
            nc.sync.dma_start(out=idx_sb[:], in_=idx_wr[:])
            drel_sb = pers.tile([128, TOT_TILES], U8)
            nc.sync.dma_start(out=drel_sb[:], in_=drel_pt[:])
            batch_sb = pers.tile([128, NW], F32)
            nc.sync.dma_start(out=batch_sb[:], in_=batch_wd[:])
            adt1_sb = pers.tile([128, NW, 4], BF16)
            adt2_sb = pers.tile([128, NW, 1], BF16)
            nidx_regs = {}
            for b in range(NW * NSHARD):
                Tb = int(tiles[b])
                for q0 in range(0, Tb, 8):
                    n = min(8, Tb - q0) * 128
                    if n not in nidx_regs:
                        nidx_regs[n] = nc.gpsimd.to_reg(n)

            # ---- phase 1: GRU -> xcat1l (h|asrc), adt1_sb ----
            with (
                tc.tile_pool(name="gx", bufs=2) as gx,
                tc.tile_pool(name="gh", bufs=2) as gh,
                tc.tile_pool(name="gv", bufs=3) as gv,
                tc.tile_pool(name="gp1", bufs=2, space="PSUM") as gp1,
                tc.tile_pool(name="gp2", bufs=1, space="PSUM") as gp2,
                tc.tile_pool(name="gp3", bufs=1, space="PSUM") as gp3,
                tc.tile_pool(name="gp4", bufs=1, space="PSUM") as gp4,
            ):
                def gru_load(it, half):
                    xt8 = gx.tile([128, T, GT], BF16, tag=f"xt8{half}")
                    for t in range(T):
                        nc.sync.dma_start(out=xt8[:, t, :],
                                          in_=xpT[t, :, it * GT:(it + 1) * GT])
                    hT = gh.tile([65, GT], BF16, tag=f"hT{half}")
                    nc.vector.memset(hT[0:64, :], 0.0)
                    nc.vector.memset(hT[64:65, :], 1.0)
                    return xt8, hT

                def gru_step(xt8, hT, t):
                    prz = gp1.tile([64, 2 * GT], F32, tag="prz")
                    nc.tensor.matmul(out=prz[:, 0:GT], lhsT=wihT_s[:, 0:64],
                                     rhs=xt8[:, t, :], start=True, stop=False)
                    nc.tensor.matmul(out=prz[:, 0:GT], lhsT=whrz_s[:, 0:64],
                                     rhs=hT[:], start=False, stop=True)
                    nc.tensor.matmul(out=prz[:, GT:2 * GT], lhsT=wihT_s[:, 64:128],
                                     rhs=xt8[:, t, :], start=True, stop=False)
                    nc.tensor.matmul(out=prz[:, GT:2 * GT], lhsT=whrz_s[:, 64:128],
                                     rhs=hT[:], start=False, stop=True)
                    pin = gp2.tile([64, GT], F32, tag="pin")
                    nc.tensor.matmul(out=pin[:], lhsT=wihT_s[:, 128:192],
                                     rhs=xt8[:, t, :], start=True, stop=True)
                    phn = gp3.tile([64, GT], F32, tag="phn")
                    nc.tensor.matmul(out=phn[:], lhsT=whn_s[:], rhs=hT[:],
                                     start=True, stop=True)
                    rz = gv.tile([64, 2 * GT], BF16, tag="rz")
                    nc.scalar.activation(out=rz[:], in_=prz[:], func=AF.Sigmoid)
                    tmp = gv.tile([64, GT], BF16, tag="tmp")
                    nc.vector.tensor_mul(out=tmp[:], in0=rz[:, 0:GT], in1=phn[:])
                    t3 = gv.tile([64, GT], F32, tag="t3")
                    nc.vector.tensor_add(out=t3[:], in0=pin[:], in1=tmp[:])
                    nh = gv.tile([64, GT], BF16, tag="nh")
                    nc.scalar.activation(out=nh[:], in_=t3[:], func=AF.Tanh,
                                         bias=bihn_s[:, 0:1])
                    s1 = gv.tile([64, GT], BF16, tag="s1")
                    nc.vector.tensor_sub(out=s1[:], in0=hT[0:64, :], in1=nh[:])
                    s2 = gv.tile([64, GT], BF16, tag="s2")
                    nc.vector.tensor_mul(out=s2[:], in0=rz[:, GT:2 * GT], in1=s1[:])
                    nc.vector.tensor_add(out=hT[0:64, :], in0=nh[:], in1=s2[:])

                def gru_out(it, hT):
                    for cc in range(GT // 128):
                        w = it * (GT // 128) + cc
                        pt = gp4.tile([128, 64], BF16, tag="pt")
                        nc.tensor.transpose(out=pt[:],
                                            in_=hT[0:64, cc * 128:(cc + 1) * 128],
                                            identity=identb[0:64, 0:64])
                        pat = gp4.tile([128, 8], F32, tag="pat")
                        nc.tensor.matmul(out=pat[:], lhsT=hT[0:64, cc * 128:(cc + 1) * 128],
                                         rhs=BC1_s[:], start=True, stop=True)
                        xc = gv.tile([128, 128], BF16, tag="xc")
                        nc.vector.memset(xc[:, 0:1], 1.0)
                        nc.scalar.copy(out=xc[:, 1:65], in_=pt[:])
                        nc.vector.tensor_copy(out=xc[:, 65:69], in_=pat[:, 0:4])
                        nc.vector.memset(xc[:, 69:128], 0.0)
                        nc.vector.tensor_copy(out=adt1_sb[:, w, :], in_=pat[:, 4:8])
                        nc.sync.dma_start(out=xcat1l[w * 128:(w + 1) * 128, :], in_=xc[:])

                for ip in range(NT // 2):
                    itA, itB = 2 * ip, 2 * ip + 1
                    xtA, hTA = gru_load(itA, 0)
                    xtB, hTB = gru_load(itB, 1)
                    for t in range(T):
                        gru_step(xtA, hTA, t)
                        gru_step(xtB, hTB, t)
                    gru_out(itA, hTA)
                    gru_out(itB, hTB)
                if NT % 2:
                    it = NT - 1
                    xtL, hTL = gru_load(it, 0)
                    for t in range(T):
                        gru_step(xtL, hTL, t)
                    gru_out(it, hTL)

            # ---- AllGather table1, split into shards ----
            nc.gpsimd.collective_compute(
                "AllGather", OP.bypass, replica_groups=rg_all,
                ins=[xcat1l[:]], outs=[table1[:]])
            for s in range(NSHARD):
                for ci in range(2):
                    nc.sync.dma_start(
                        out=tb1s[s][ci * NPAD2:(ci + 1) * NPAD2, :],
                        in_=table1[(2 * s + ci) * NPAD2:(2 * s + ci + 1) * NPAD2, :])

            # ---- GAT layer over windows (shared for layer 1 / layer 2) ----
            def gat_windows(tbls, adt_sb, nheads, payw, post_fn):
                """payw: scatter matmul width (4+256 for L1, 1+64 for L2)."""
                with (
                    tc.tile_pool(name="pg", bufs=3) as pg,
                    tc.tile_pool(name="pS", bufs=2) as pS,
                    tc.tile_pool(name="pd", bufs=2) as pd,
                    tc.tile_pool(name="pu", bufs=2) as pu,
                    tc.tile_pool(name="pM", bufs=2) as pM,
                    tc.tile_pool(name="pw", bufs=2) as pw,
                    tc.tile_pool(name="ps2", bufs=2, space="PSUM") as ps2,
                    tc.tile_pool(name="psA", bufs=1, space="PSUM") as psA,
                    tc.tile_pool(name="psB", bufs=1, space="PSUM") as psB,
                ):
                    for w in range(NW):
                        pblk = ps2.tile([128, payw], F32, tag="pblk")
                        first = True
                        for s in range(NSHARD):
                            b = w * NSHARD + s
                            Tb = int(tiles[b])
                            t0 = int(tile_off[b])
                            g = pg.tile([128, Tb, 128], BF16, tag=f"g{Tb}")
                            for q0 in range(0, Tb, 8):
                                qn = min(8, Tb - q0)
                                nc.gpsimd.dma_gather(
                                    out_ap=g[:, q0:q0 + qn, :], in_ap=tbls[s][:],
                                    idxs_ap=idx_sb[:, (t0 + q0) * 8:(t0 + q0 + qn) * 8],
                                    num_idxs=qn * 128, num_idxs_reg=nidx_regs[qn * 128],
                                    elem_size=128)
                            S = pS.tile([128, Tb, 128], BF16, tag=f"S{Tb}")
                            nc.vector.tensor_tensor(
                                out=S[:],
                                in0=iota_row[:].unsqueeze(1).broadcast_to([128, Tb, 128]),
                                in1=drel_sb[:, t0:t0 + Tb].unsqueeze(2).broadcast_to([128, Tb, 128]),
                                op=OP.is_equal)
                            drT = pd.tile([128, Tb, 128], U8, tag=f"dT{Tb}")
                            nc.sync.dma_start(
                                out=drT[:],
                                in_=drelT[0:1, t0:t0 + Tb, :].partition_broadcast(128))
                            Sd = pS.tile([128, Tb, 128], BF16, tag=f"Sd{Tb}")
                            nc.vector.tensor_scalar(
                                out=Sd[:], in0=drT[:], scalar1=iota_p[:, 0:1],
                                scalar2=None, op0=OP.is_equal)
                            padp = psA.tile([128, Tb * nheads], F32, tag="padp")
                            for t in range(Tb):
                                nc.tensor.matmul(
                                    out=padp[:, t * nheads:(t + 1) * nheads],
                                    lhsT=Sd[:, t, :], rhs=adt_sb[:, w, :],
                                    start=True, stop=True)
                            u = pu.tile([128, Tb, nheads], F32, tag=f"u{Tb}")
                            nc.vector.tensor_add(
                                out=u[:], in0=g[:, :, 65:65 + nheads],
                                in1=padp[:].rearrange("p (t c) -> p t c", t=Tb))
                            e1 = pu.tile([128, Tb, nheads], BF16, tag=f"e1{Tb}")
                            nc.scalar.activation(out=e1[:], in_=u[:], func=AF.Exp)
                            e2 = pu.tile([128, Tb, nheads], BF16, tag=f"e2{Tb}")
                            nc.scalar.activation(out=e2[:], in_=u[:], func=AF.Exp, scale=0.2)
                            ee = pu.tile([128, Tb, nheads], BF16, tag=f"ee{Tb}")
                            nc.vector.tensor_tensor(out=ee[:], in0=e1[:], in1=e2[:], op=OP.max)
                            M = pM.tile([128, Tb, payw], BF16, tag=f"M{Tb}")
                            if nheads == 4:
                                nc.vector.tensor_tensor(
                                    out=M[:].rearrange("p t (h c) -> p t h c", h=4),
                                    in0=g[:, :, 0:65].unsqueeze(2).broadcast_to([128, Tb, 4, 65]),
                                    in1=ee[:].unsqueeze(3).broadcast_to([128, Tb, 4, 65]),
                                    op=OP.mult)
                            else:
                                nc.vector.tensor_tensor(
                                    out=M[:],
                                    in0=g[:, :, 0:65],
                                    in1=ee[:].to_broadcast([128, Tb, 65]),
                                    op=OP.mult)
                            for t in range(Tb):
                                nc.tensor.matmul(
                                    out=pblk[:], lhsT=S[:, t, :], rhs=M[:, t, :],
                                    start=first, stop=(s == NSHARD - 1 and t == Tb - 1))
                                first = False
                        post_fn(w, pblk, pw, psB)

            # ---- layer 1 post: h1 = relu(z/denom @ W1 + b1) -> xcat2, adt2 ----
            def post1(w, pblk, pw, psB):
                dn = pw.tile([128, 4], F32, tag="dn")
                nc.vector.tensor_scalar(
                    out=dn[:], in0=pblk[:].rearrange("p (h c) -> p h c", h=4)[:, :, 0],
                    scalar1=1e-12, scalar2=None, op0=OP.max)
                rec = pw.tile([128, 4], F32, tag="rec")
                nc.vector.reciprocal(out=rec[:], in_=dn[:])
                zn = pw.tile([128, 256], BF16, tag="zn")
                nc.vector.tensor_tensor(
                    out=zn[:].rearrange("p (h c) -> p h c", h=4),
                    in0=pblk[:].rearrange("p (h c) -> p h c", h=4)[:, :, 1:65],
                    in1=rec[:].unsqueeze(2).broadcast_to([128, 4, 64]),
                    op=OP.mult)
                tpz = pw.tile([128, 256], BF16, tag="tpz")
                for half in range(2):
                    ptp = psB.tile([128, 128], BF16, tag="ptp")
                    nc.tensor.transpose(out=ptp[:], in_=zn[:, 128 * half:128 * (half + 1)],
                                        identity=identb[:])
                    nc.scalar.copy(out=tpz[:, 128 * half:128 * (half + 1)], in_=ptp[:])
                h1p = psB.tile([128, 256], F32, tag="h1p")
                nc.tensor.matmul(out=h1p[:, 0:128], lhsT=tpz[:, 0:128], rhs=W12A_s[:],
                                 start=True, stop=True)
                nc.tensor.matmul(out=h1p[:, 128:256], lhsT=tpz[:, 128:256], rhs=W12B_s[:],
                                 start=True, stop=True)
                h1 = pw.tile([128, 256], BF16, tag="h1")
                nc.vector.tensor_add(out=h1[:], in0=h1p[:], in1=b1b_s[:])
                nc.vector.tensor_scalar(out=h1[:], in0=h1[:], scalar1=0.0,
                                        scalar2=None, op0=OP.max)
                th1 = pw.tile([128, 256], BF16, tag="th1")
                for half in range(2):
                    ptp = psB.tile([128, 128], BF16, tag="ptp")
                    nc.tensor.transpose(out=ptp[:], in_=h1[:, 128 * half:128 * (half + 1)],
                                        identity=identb[:])
                    nc.scalar.copy(out=th1[:, 128 * half:128 * (half + 1)], in_=ptp[:])
                xsc = psB.tile([128, 66], F32, tag="xsc")
                xs2p = xsc[:, 0:64]
                pat2 = xsc[:, 64:66]
                for half in range(2):
                    nc.tensor.matmul(out=xs2p, lhsT=th1[:, 128 * half:128 * (half + 1)],
                                     rhs=W2s_s[:, 64 * half:64 * (half + 1)],
                                     start=(half == 0), stop=(half == 1))
                    nc.tensor.matmul(out=pat2, lhsT=th1[:, 128 * half:128 * (half + 1)],
                                     rhs=BC2_s[:, 2 * half:2 * half + 2],
                                     start=(half == 0), stop=(half == 1))
                xc2 = pw.tile([128, 128], BF16, tag="xc2")
                nc.vector.memset(xc2[:, 0:1], 1.0)
                nc.scalar.copy(out=xc2[:, 1:65], in_=xs2p)
                nc.vector.tensor_copy(out=xc2[:, 65:66], in_=pat2[:, 0:1])
                nc.vector.memset(xc2[:, 66:128], 0.0)
                nc.vector.tensor_copy(out=adt2_sb[:, w, :], in_=pat2[:, 1:2])
                nc.sync.dma_start(out=xcat2l[w * 128:(w + 1) * 128, :], in_=xc2[:])

            gat_windows(tb1s, adt1_sb, 4, 260, post1)

            nc.gpsimd.collective_compute(
                "AllGather", OP.bypass, replica_groups=rg_all,
                ins=[xcat2l[:]], outs=[table2[:]])
            for s in range(NSHARD):
                for ci in range(2):
                    nc.sync.dma_start(
                        out=tb2s[s][ci * NPAD2:(ci + 1) * NPAD2, :],
                        in_=table2[(2 * s + ci) * NPAD2:(2 * s + ci + 1) * NPAD2, :])

            # ---- layer 2 post: pooling into ppool psum ----
            pp_ctx = tc.tile_pool(name="psPool", bufs=1, space="PSUM")
            psPool = pp_ctx.__enter__()
            ppool = psPool.tile([128, 65], F32)

            def post2(w, pblk, pw, psB):
                dn2 = pw.tile([128, 1], F32, tag="dn2")
                nc.vector.tensor_scalar(out=dn2[:], in0=pblk[:, 0:1], scalar1=1e-12,
                                        scalar2=None, op0=OP.max)
                rec2 = pw.tile([128, 1], F32, tag="rec2")
                nc.vector.reciprocal(out=rec2[:], in_=dn2[:])
                ph = pw.tile([128, 65], BF16, tag="ph")
                nc.vector.tensor_scalar(out=ph[:, 0:64], in0=pblk[:, 1:65],
                                        scalar1=rec2[:, 0:1], scalar2=None, op0=OP.mult)
                nc.vector.tensor_add(out=ph[:, 0:64], in0=ph[:, 0:64], in1=b2b_s[:])
                nc.vector.tensor_scalar(out=ph[:, 0:64], in0=ph[:, 0:64], scalar1=0.0,
                                        scalar2=None, op0=OP.max)
                nc.vector.memset(ph[:, 64:65], 1.0)
                Sb = pw.tile([128, 128], BF16, tag="Sb")
                nc.vector.tensor_scalar(out=Sb[:], in0=iota_row[:],
                                        scalar1=batch_sb[:, w:w + 1], scalar2=None,
                                        op0=OP.is_equal)
                nc.tensor.matmul(out=ppool[:], lhsT=Sb[:], rhs=ph[:],
                                 start=(w == 0), stop=(w == NW - 1))

            gat_windows(tb2s, adt2_sb, 1, 65, post2)

            # ---- tail: AllReduce pools, fc, log_softmax ----
            with tc.tile_pool(name="rpre", bufs=1) as rpre:
                pr = rpre.tile([128, 65], F32)
                nc.scalar.copy(out=pr[:], in_=ppool[:])
                nc.sync.dma_start(out=arin[:], in_=pr[:])
            pp_ctx.__exit__(None, None, None)
            with (
                tc.tile_pool(name="r5", bufs=1) as r5,
                tc.tile_pool(name="r5q", bufs=1, space="PSUM") as r5q,
            ):
                tc.strict_bb_all_engine_barrier()
                nc.gpsimd.collective_compute(
                    "AllReduce", OP.add, replica_groups=rg_all,
                    ins=[arin[:]], outs=[arout[:]])
                tc.strict_bb_all_engine_barrier()
                ar = r5.tile([128, 65], F32)
                nc.sync.dma_start(out=ar[:], in_=arout[:])
                cm = r5.tile([128, 1], F32)
                nc.vector.tensor_scalar(out=cm[:], in0=ar[:, 64:65], scalar1=1.0,
                                        scalar2=None, op0=OP.max)
                cr = r5.tile([128, 1], F32)
                nc.vector.reciprocal(out=cr[:], in_=cm[:])
                gf = r5.tile([128, 64], F32)
                nc.vector.tensor_scalar(out=gf[:], in0=ar[:, 0:64], scalar1=cr[:, 0:1],
                                        scalar2=None, op0=OP.mult)
                identf = r5.tile([128, 128], F32)
                make_identity(nc, identf[:])
                pgt = r5q.tile([64, 128], F32)
                nc.tensor.transpose(out=pgt[:], in_=gf[:], identity=identf[:])
                gfT = r5.tile([64, 128], F32)
                nc.scalar.copy(out=gfT[:], in_=pgt[:])
                plg = r5q.tile([128, 10], F32)
                nc.tensor.matmul(out=plg[:], lhsT=gfT[:], rhs=fcw_s[:], start=True, stop=True)
                lg = r5.tile([128, 16], F32)
                nc.vector.tensor_add(out=lg[:, 0:10], in0=plg[:], in1=fcbb_s[:])
                mx = r5.tile([128, 1], F32)
                nc.vector.reduce_max(out=mx[:], in_=lg[:, 0:10], axis=mybir.AxisListType.X)
                tsh = r5.tile([128, 16], F32)
                nc.vector.tensor_scalar(out=tsh[:, 0:10], in0=lg[:, 0:10],
                                        scalar1=mx[:, 0:1], scalar2=None, op0=OP.subtract)
                exs = r5.tile([128, 16], F32)
                se = r5.tile([128, 1], F32)
                nc.scalar.activation(out=exs[:, 0:10], in_=tsh[:, 0:10], func=AF.Exp,
                                     accum_out=se[:])
                ln = r5.tile([128, 1], F32)
                nc.scalar.activation(out=ln[:], in_=se[:], func=AF.Ln)
                res = r5.tile([128, 16], F32)
                nc.vector.memset(res[:], 0.0)
                nc.vector.tensor_scalar(out=res[:, 0:10], in0=tsh[:, 0:10],
                                        scalar1=ln[:, 0:1], scalar2=None, op0=OP.subtract)
                nc.sync.dma_start(out=out[:], in_=res[:])
    nc.compile()
    return nc


# ---------------- self-contained entry point ----------------
_CACHE = {}


def kernel(**inputs):
    """Full DAGNN forward. Takes the unsharded inputs from setup_inputs();
    returns log-softmax output [num_graphs, 10] float32."""
    x = np.asarray(inputs["x"], np.float32)
    edge_index = np.asarray(inputs["edge_index"])
    batch = np.asarray(inputs["batch"])
    G = int(inputs["num_graphs"])
    weights = [np.asarray(inputs[k], np.float32) for k in (
        "gru_w_ih", "gru_w_hh", "gru_b_ih", "gru_b_hh",
        "W1", "att_src1", "att_dst1", "b1",
        "W2", "att_src2", "att_dst2", "b2", "fc_w", "fc_b")]
    N = x.shape[0]
    E = edge_index.shape[1]
    P = 8

    from concourse.bass_utils import run_bass_kernel_spmd
    cfg = Cfg(N, E, G, P)
    per_core = host_prep(cfg, edge_index, batch)
    in_maps = build_inputs(cfg, x, weights, per_core)
    key = (N, E, G, P, cfg.NPAD2, cfg.TOT_TILES, tuple(cfg.tiles[:8]))
    if key not in _CACHE:
        _CACHE[key] = build_kernel(cfg)
    nc = _CACHE[key]
    res = run_bass_kernel_spmd(nc, in_maps, core_ids=list(range(P)))
    out = np.asarray(res.results[0]["out"][:G, :10], np.float32)
    return out
